# revision 1
# baseline (speedup 1.0000x reference)
"""Self-contained Trainium2 Bass kernel for BoSs (block-of-states) attention.

Strategy (8 NeuronCores):
  - data-parallel over batch (2) x tensor-parallel over heads (4):
    core c handles batch c//4, q-heads [4g:4g+4] and kv-head g where g=c%4.
  - host packs tokens by state id (stable sort) so the BoSs mask becomes
    block-banded causal in packed coordinates (max segment ~280 << WIN=1024,
    so the sliding window never binds and a 3-block lookback band suffices).
  - transposed activation layouts ([dim, seq]) keep every matmul contracting
    over the partition axis; scores are computed transposed ([k, q]) so the
    attention-weights matrix feeds the AV matmul without transposes.
  - fp16 matmul operands (full PE rate + FWL), fp32 PSUM accumulation.
  - additive {0,-30000} mask applied to scores in fp32 PSUM before exp, so
    no max-subtraction is needed (scores are bounded) and masked scores
    never overflow the fp16 attention-weight tiles.
  - softmax denominator via a ones-vector matmul; 1/l broadcast across
    partitions via a K=1 matmul; partial row-parallel Wo outputs are summed
    and unpermuted on host.
"""

import numpy as np
from contextlib import ExitStack

# problem constants (hardcoded per spec)
B, L, HID = 2, 2048, 2048
H, KVH, D = 16, 4, 128
THETA = 10000.0
NCORES = 8
TP = 4            # tensor-parallel group size (cores per batch)
QH = H // TP      # q heads per core = 4
QCH = 256         # q columns per attention chunk
NJQ = L // QCH    # 8
NKB = L // 128    # 16 k-blocks / q-blocks
NHC = HID // 128  # 16 hidden-dim chunks
LC = 512          # phase-1 L-chunk width
NLC = L // LC     # 4
BAND_BACK = 3     # k-block lookback; correct while max segment <= 385
SCALE = float(D) ** -0.5
# uniform logit shift (in raw-score units) folded into the additive mask;
# keeps exp() outputs comfortably inside fp16 range and cancels exactly in
# the softmax normalization (same constant for every valid entry).
MASK_SHIFT = -2.0 / SCALE
MASK_NEG = -30000.0


def _band(jq):
    lo = max(0, 2 * jq - BAND_BACK)
    hi = 2 * jq + 1
    return lo, hi


_BANDS = [_band(j) for j in range(NJQ)]
_NBLK = sum(hi - lo + 1 for lo, hi in _BANDS)
_MBASE = np.cumsum([0] + [hi - lo + 1 for lo, hi in _BANDS]).tolist()

_CACHE = {}
LAST_EXEC_NS = None
LAST_RUN_WALL_S = None


def _build_nc():
    import concourse.tile as tile
    from concourse import bacc, mybir

    f32 = mybir.dt.float32
    f16 = mybir.dt.float16
    EXP = mybir.ActivationFunctionType.Exp

    nc = bacc.Bacc(
        "TRN2", target_bir_lowering=False, debug=False, num_devices=NCORES
    )

    xT = nc.dram_tensor("xT", [HID, L], f16, kind="ExternalInput").ap()
    wq = nc.dram_tensor("wq", [HID, QH * D], f16, kind="ExternalInput").ap()
    wk = nc.dram_tensor("wk", [HID, D], f16, kind="ExternalInput").ap()
    wv = nc.dram_tensor("wv", [HID, D], f16, kind="ExternalInput").ap()
    wo = nc.dram_tensor("wo", [QH * D, HID], f16, kind="ExternalInput").ap()
    cosd = nc.dram_tensor("cosd", [D, L], f16, kind="ExternalInput").ap()
    sind = nc.dram_tensor("sind", [D, L], f16, kind="ExternalInput").ap()
    mskd = nc.dram_tensor("mskd", [_NBLK, 128, QCH], f16, kind="ExternalInput").ap()
    swpd = nc.dram_tensor("swpd", [128, 128], f16, kind="ExternalInput").ap()
    idnd = nc.dram_tensor("idnd", [128, 128], f16, kind="ExternalInput").ap()
    out = nc.dram_tensor("out", [L, HID], f16, kind="ExternalOutput").ap()

    with tile.TileContext(nc) as tc, ExitStack() as top:
        persist = top.enter_context(tc.tile_pool(name="persist", bufs=1))
        kT = persist.tile([128, L], f16, tag="kT", name="kT")
        qT = [
            persist.tile([128, L], f16, tag=f"qT{h}", name=f"qT{h}")
            for h in range(QH)
        ]
        oT = [
            persist.tile([128, L], f16, tag=f"oT{h}", name=f"oT{h}")
            for h in range(QH)
        ]
        vA = persist.tile([128, NKB, 128], f16, tag="vA", name="vA")
        cosT = persist.tile([128, L], f16, tag="cosT", name="cosT")
        sinT = persist.tile([128, L], f16, tag="sinT", name="sinT")
        ones = persist.tile([128, 128], f16, tag="ones", name="ones")
        swp = persist.tile([128, 128], f16, tag="swp", name="swp")
        idn = persist.tile([128, 128], f16, tag="idn", name="idn")

        nc.any.memset(ones[:], 1.0)

        # weights / inputs (live whole kernel; everything coexists so the
        # scheduler can overlap phases by data deps alone)
        wpool = top.enter_context(tc.tile_pool(name="wpool", bufs=1))
        wq_s = wpool.tile([128, NHC, QH * D], f16, tag="wq", name="wq_s")
        wk_s = wpool.tile([128, NHC, D], f16, tag="wk", name="wk_s")
        wv_s = wpool.tile([128, NHC, D], f16, tag="wv", name="wv_s")
        vT_s = wpool.tile([128, L], f16, tag="vT", name="vT_s")
        wo_s = wpool.tile([128, QH, HID], f16, tag="wo", name="wo_s")
        xpool = top.enter_context(tc.tile_pool(name="xpool", bufs=2))

        # DMA emission order = first-needed-first: k/v weights + x chunk 0
        # interleaved, then q weights, then rope tables; wo after phase 1.
        # Early DMAs are spread over four sequencers: single-queue issue is
        # ~650ns per dma_start, which would serialize the startup stream.
        qs_engines = [nc.sync, nc.scalar, nc.gpsimd]
        xt0 = xpool.tile([128, NHC, LC], f16, tag="x", name="xt0")
        for c in range(NHC):
            eng = qs_engines[c % 3]
            eng.dma_start(wk_s[:, c, :], wk[c * 128 : (c + 1) * 128, :])
            eng.dma_start(xt0[:, c, :], xT[c * 128 : (c + 1) * 128, 0:LC])
            eng.dma_start(wv_s[:, c, :], wv[c * 128 : (c + 1) * 128, :])
            eng.dma_start(wq_s[:, c, :], wq[c * 128 : (c + 1) * 128, :])
        nc.scalar.dma_start(swp[:], swpd[:])
        nc.sync.dma_start(cosT[:], cosd[:])
        nc.sync.dma_start(sinT[:], sind[:])
        nc.gpsimd.dma_start(idn[:], idnd[:])
        tpool = top.enter_context(tc.tile_pool(name="tpool", bufs=3))
        mpool = top.enter_context(tc.tile_pool(name="mpool", bufs=2))
        ppool = top.enter_context(tc.tile_pool(name="ppool", bufs=2))
        spool = top.enter_context(tc.tile_pool(name="spool", bufs=2))
        # PSUM: 8 banks total. big([128,512] f32 = 1 bank) x3 for
        # projections/swap/vtr/final; S([128,3,256] f32 = 2 banks) x2;
        # o(1 bank) x1; l(1 bank) x1.
        psB = top.enter_context(tc.tile_pool(name="psB", bufs=2, space="PSUM"))
        psS = top.enter_context(tc.tile_pool(name="psS", bufs=2, space="PSUM"))
        psO = top.enter_context(tc.tile_pool(name="psO", bufs=1, space="PSUM"))
        psL = top.enter_context(tc.tile_pool(name="psL", bufs=1, space="PSUM"))

        # ---- phase 1: projections (qT/kT rope'd, v transposed) ----
        for lc in range(NLC):
            cols = slice(lc * LC, (lc + 1) * LC)
            if lc == 0:
                xt = xt0
            else:
                xt = xpool.tile([128, NHC, LC], f16, tag="x", name=f"xt{lc}")
                for c in range(NHC):
                    eng = nc.sync if c % 2 == 0 else nc.gpsimd
                    eng.dma_start(
                        xt[:, c, :], xT[c * 128 : (c + 1) * 128, cols]
                    )
            # k first: its (small) weights arrive earliest, so PE starts sooner
            for hb in (QH, QH + 1, 0, 1, 2, 3):
                ps = psB.tile([128, LC], f32, tag="big", name=f"ps{lc}_{hb}")
                for c in range(NHC):
                    if hb < QH:
                        lhsT = wq_s[:, c, hb * 128 : (hb + 1) * 128]
                    elif hb == QH:
                        lhsT = wk_s[:, c, :]
                    else:
                        lhsT = wv_s[:, c, :]
                    nc.tensor.matmul(
                        ps[:],
                        lhsT,
                        xt[:, c, :],
                        start=(c == 0),
                        stop=(c == NHC - 1),
                    )
                if hb <= QH:  # rope for q & k
                    dst = qT[hb] if hb < QH else kT
                    plain = tpool.tile(
                        [128, LC], f16, tag="plain", name=f"pl{lc}_{hb}"
                    )
                    nc.scalar.copy(plain[:], ps[:])
                    sw = psB.tile([128, LC], f32, tag="big", name=f"sw{lc}_{hb}")
                    nc.tensor.matmul(sw[:], swp[:], plain[:], start=True, stop=True)
                    t1 = tpool.tile([128, LC], f16, tag="t1", name=f"t1_{lc}_{hb}")
                    nc.gpsimd.tensor_mul(t1[:], plain[:], cosT[:, cols])
                    t2 = tpool.tile([128, LC], f16, tag="t2", name=f"t2_{lc}_{hb}")
                    nc.vector.tensor_mul(t2[:], sw[:], sinT[:, cols])
                    nc.gpsimd.tensor_add(dst[:, cols], t1[:], t2[:])
                else:  # v: keep transposed copy, then transpose this chunk
                    nc.scalar.copy(vT_s[:, cols], ps[:])
                    for kb in range(lc * (LC // 128), (lc + 1) * (LC // 128)):
                        vt_ps = psB.tile(
                            [128, 128], f32, tag="big", name=f"vt{kb}"
                        )
                        nc.tensor.matmul(
                            vt_ps[:],
                            vT_s[:, kb * 128 : (kb + 1) * 128],
                            idn[:],
                            start=True,
                            stop=True,
                        )
                        nc.scalar.copy(vA[:, kb, :], vt_ps[:])

        nc.sync.dma_start(wo_s[:], wo.rearrange("(h p) n -> p h n", p=128))

        # ---- phase 2: banded attention in transposed layout ----
        SB = 3  # S sub-chunk width in k-blocks (2 PSUM banks)
        for jq in range(NJQ):
            lo, hi = _BANDS[jq]
            nkb = hi - lo + 1
            qs = slice(jq * QCH, (jq + 1) * QCH)
            msk = mpool.tile([128, nkb, QCH], f16, tag="m", name=f"msk{jq}")
            nc.sync.dma_start(
                msk[:],
                mskd[_MBASE[jq] : _MBASE[jq] + nkb].rearrange("k p n -> p k n"),
            )
            for h in range(QH):
                P = ppool.tile([128, nkb, QCH], f16, tag="P", name=f"p{jq}_{h}")
                for p0 in range(0, nkb, SB):
                    pn = min(SB, nkb - p0)
                    s_ps = psS.tile(
                        [128, SB, QCH], f32, tag="S", name=f"s{jq}_{h}_{p0}"
                    )
                    for i in range(p0, p0 + pn):
                        kb = lo + i
                        nc.tensor.matmul(
                            s_ps[:, i - p0, :],
                            kT[:, kb * 128 : (kb + 1) * 128],
                            qT[h][:, qs],
                            start=True,
                            stop=True,
                        )
                    # additive mask in fp32 PSUM (in-place), then exp -> fp16
                    nc.vector.tensor_add(
                        s_ps[:, :pn, :], s_ps[:, :pn, :], msk[:, p0 : p0 + pn, :]
                    )
                    nc.scalar.activation(
                        P[:, p0 : p0 + pn, :], s_ps[:, :pn, :], EXP, scale=SCALE
                    )
                l_ps = psL.tile([1, QCH], f32, tag="l", name=f"l{jq}_{h}")
                for i in range(nkb):
                    nc.tensor.matmul(
                        l_ps[:],
                        ones[:, 0:1],
                        P[:, i, :],
                        start=(i == 0),
                        stop=(i == nkb - 1),
                    )
                o_ps = psO.tile([128, QCH], f32, tag="o", name=f"o{jq}_{h}")
                for i in range(nkb):
                    kb = lo + i
                    nc.tensor.matmul(
                        o_ps[:],
                        vA[:, kb, :],
                        P[:, i, :],
                        start=(i == 0),
                        stop=(i == nkb - 1),
                    )
                rc = spool.tile([1, QCH], f16, tag="lsb", name=f"ls{jq}_{h}")
                with nc.allow_low_precision(
                    reason="fp16 1/l scales fp16 outputs; 5e-4 rel ok"
                ):
                    nc.vector.reciprocal(rc[:], l_ps[:])
                r_bc = spool.tile([128, QCH], f16, tag="lbc", name=f"lb{jq}_{h}")
                nc.gpsimd.partition_broadcast(r_bc[:], rc[:])
                nc.vector.tensor_mul(oT[h][:, qs], o_ps[:], r_bc[:])

        # ---- phase 3: output projection (row-parallel partial) ----
        for qb in range(NKB):
            for hc in range(HID // 512):
                # late groups borrow the attention pool's idle banks so the
                # PSUM->SBUF copy isn't on the matmul critical path
                if qb >= 10 and (qb * 4 + hc) % 2 == 0:
                    f_ps = psS.tile(
                        [128, 512], f32, tag="S", name=f"f{qb}_{hc}"
                    )
                else:
                    f_ps = psB.tile(
                        [128, 512], f32, tag="big", name=f"f{qb}_{hc}"
                    )
                for h in range(QH):
                    nc.tensor.matmul(
                        f_ps[:],
                        oT[h][:, qb * 128 : (qb + 1) * 128],
                        wo_s[:, h, hc * 512 : (hc + 1) * 512],
                        start=(h == 0),
                        stop=(h == QH - 1),
                    )
                ob = spool.tile(
                    [128, 512], f16, tag="ob", bufs=4, name=f"ob{qb}_{hc}"
                )
                nc.any.tensor_copy(ob[:], f_ps[:])
                nc.sync.dma_start(
                    out[qb * 128 : (qb + 1) * 128, hc * 512 : (hc + 1) * 512],
                    ob[:],
                )

    nc.compile()
    return nc


def _get_nc():
    if "nc" not in _CACHE:
        _CACHE["nc"] = _build_nc()
    return _CACHE["nc"]


def kernel(hidden_states, Wq, Wk, Wv, Wo, sid, position_ids):
    global LAST_EXEC_NS, LAST_RUN_WALL_S
    import time

    from concourse.bass_utils import run_bass_kernel_spmd

    hidden = np.asarray(hidden_states, dtype=np.float32)
    Wq = np.asarray(Wq, dtype=np.float32)
    Wk = np.asarray(Wk, dtype=np.float32)
    Wv = np.asarray(Wv, dtype=np.float32)
    Wo = np.asarray(Wo, dtype=np.float32)
    sid = np.asarray(sid)
    position_ids = np.asarray(position_ids)

    nc = _get_nc()

    f16 = np.float16
    swp = np.zeros((128, 128), f16)
    swp[(np.arange(128) + 64) % 128, np.arange(128)] = 1.0
    idn = np.eye(128, dtype=f16)

    in_maps = []
    perms = []
    for b in range(B):
        s = sid[b].astype(np.int64)
        perm = np.argsort(s, kind="stable")
        perms.append(perm)
        st = s[perm]
        seg_max = int(np.bincount(st, minlength=1).max())
        assert seg_max <= BAND_BACK * 128 + 1, (
            f"segment length {seg_max} exceeds supported band"
        )

        pos = position_ids[b][perm].astype(np.float32)
        inv = (
            1.0
            / (THETA ** (np.arange(0, D, 2, dtype=np.float32) / np.float32(D)))
        ).astype(np.float32)
        fr = pos[:, None] * inv[None, :]
        emb = np.concatenate([fr, fr], axis=1)  # [L, D]
        cosT = np.ascontiguousarray(np.cos(emb).T.astype(f16))
        sinT = np.sin(emb).T.astype(np.float32).copy()
        sinT[: D // 2] *= -1.0  # fold rotate_half sign
        sinT = np.ascontiguousarray(sinT.astype(f16))

        xTp = np.ascontiguousarray(hidden[b].T[:, perm].astype(f16))

        msk = np.full((_NBLK, 128, QCH), MASK_NEG, f16)
        ki = np.arange(128)
        qi = np.arange(QCH)
        for jq in range(NJQ):
            lo, hi = _BANDS[jq]
            for i in range(hi - lo + 1):
                kb = lo + i
                kidx = kb * 128 + ki
                qidx = jq * QCH + qi
                m = (st[kidx][:, None] == st[qidx][None, :]) & (
                    kidx[:, None] <= qidx[None, :]
                )
                msk[_MBASE[jq] + i] = np.where(m, MASK_SHIFT, MASK_NEG).astype(f16)

        for g in range(TP):
            in_maps.append(
                dict(
                    xT=xTp,
                    wq=np.ascontiguousarray(Wq[g * 512 : (g + 1) * 512].T.astype(f16)),
                    wk=np.ascontiguousarray(Wk[g * 128 : (g + 1) * 128].T.astype(f16)),
                    wv=np.ascontiguousarray(Wv[g * 128 : (g + 1) * 128].T.astype(f16)),
                    wo=np.ascontiguousarray(
                        Wo[:, g * 512 : (g + 1) * 512].T.astype(f16)
                    ),
                    cosd=cosT,
                    sind=sinT,
                    mskd=msk,
                    swpd=swp,
                    idnd=idn,
                )
            )

    t0 = time.time()
    res = run_bass_kernel_spmd(nc, in_maps, core_ids=list(range(NCORES)))
    LAST_RUN_WALL_S = time.time() - t0
    LAST_EXEC_NS = res.exec_time_ns

    full = np.empty((B, L, HID), np.float32)
    for b in range(B):
        acc = np.asarray(res.results[4 * b]["out"]).astype(np.float32)
        for g in range(1, TP):
            acc += np.asarray(res.results[4 * b + g]["out"]).astype(np.float32)
        unp = np.empty_like(acc)
        unp[perms[b]] = acc
        full[b] = unp
    return full



# revision 9
# speedup vs baseline: 1.0241x; 1.0241x over previous
"""Self-contained Trainium2 Bass kernel for BoSs (block-of-states) attention.

Strategy (8 NeuronCores):
  - data-parallel over batch (2) x tensor-parallel over heads (4):
    core c handles batch c//4, q-heads [4g:4g+4] and kv-head g where g=c%4.
  - host sorts tokens by state id with states relabeled by descending segment
    length (so both batches produce the same padded block structure), then
    pads each segment to a multiple of 128.  In padded coordinates the BoSs
    mask is exactly: blocks within one segment, causal, with a single shared
    lower-triangle mask on diagonal blocks (plus a per-segment tail mask on
    the segment's last block).  The sliding window (1024) never binds since
    segments are ~280 tokens.
  - projections and the output GEMM run on the n-hat-packed (unpadded) token
    axis so no FLOPs are spent on padding.
  - fp8 (e4m3) DoubleRow matmuls with hi+lo error compensation for the q/k/v
    projections and the Wo GEMM: x = xh+xl, W = Wh+Wl (host-split after
    scaling into e4m3's sweet spot); the three cross terms xh*Wh, xh*Wl,
    xl*Wh are computed with paired-k-tile DoubleRow instructions (2 k-tiles
    per instruction at 0.5 cycles/row) -> 1.33x over fp16 at ~1e-3 accuracy.
  - attention (scores, softmax denominator, AV) stays fp16: its contraction
    depth (128) is too short for the pairing to pay for the extra casts.
  - global scales (inputs *8, weights *512) keep every fp8 split well above
    the e4m3 subnormal floor; the exp() activation scale and a final host
    divide undo them exactly.
"""

import numpy as np
from contextlib import ExitStack

# problem constants (hardcoded per spec)
B, L, HID = 2, 2048, 2048
H, KVH, D = 16, 4, 128
THETA = 10000.0
NCORES = 8
TP = 4            # tensor-parallel group size (cores per batch)
QH = H // TP      # q heads per core = 4
NHC = HID // 128  # 16 hidden-dim chunks
NSEG = 8
SCALE = float(D) ** -0.5

# fp8 scaling: values ~N(0, 8..10) sit mid-range in e4m3 so the hi/lo split
# residuals stay far above the subnormal floor (2^-9).
SX = 8.0
SW = 512.0
SWO = 512.0
V0 = 128.0        # folded into the softmax-denominator ones vector: oT = o/V0
SCALE_EFF = SCALE / (SW * SX) ** 2
OUT_DESCALE = V0 / (SWO * SW * SX)

_CACHE = {}
LAST_EXEC_NS = None
LAST_RUN_WALL_S = None


def _structure(sid):
    """Shared padded block structure from both batches' state histograms."""
    counts = []
    perms = []
    for b in range(B):
        s = np.asarray(sid[b]).astype(np.int64)
        n = np.bincount(s, minlength=NSEG)
        order = np.argsort(-n, kind="stable")       # states by length desc
        rank = np.empty(NSEG, np.int64)
        rank[order] = np.arange(NSEG)
        perm = np.argsort(rank[s], kind="stable")   # tokens by (rank, pos)
        counts.append(np.sort(n)[::-1])
        perms.append(perm)
    nhat = np.maximum(counts[0], counts[1])
    T = np.maximum(1, np.ceil(nhat / 128).astype(np.int64))
    assert nhat.max() <= 512, f"segment too long: {nhat.max()}"
    assert T.max() <= 4
    return tuple(int(t) for t in T), tuple(int(v) for v in nhat), perms, counts


def _build_nc(T, nhat):
    import concourse.tile as tile
    from concourse import bacc, mybir

    f32 = mybir.dt.float32
    f16 = mybir.dt.float16
    f8 = mybir.dt.float8e4
    EXP = mybir.ActivationFunctionType.Exp
    DR = mybir.MatmulPerfMode.DoubleRow

    NBLK = sum(T)
    LPAD = 128 * NBLK
    NPACK = int(sum(nhat))
    NT = (NPACK + 127) // 128          # Wo token tiles
    pbase = np.cumsum([0] + list(T)).tolist()
    nbase = np.cumsum([0] + list(nhat)).tolist()
    NMASK = 1 + NSEG

    nc = bacc.Bacc(
        "TRN2", target_bir_lowering=False, debug=False, num_devices=NCORES
    )

    x8h_d = nc.dram_tensor("x8h", [128, NHC, NPACK], f8, kind="ExternalInput").ap()
    x8l_d = nc.dram_tensor("x8l", [128, NHC, NPACK], f8, kind="ExternalInput").ap()
    wq8_d = [
        nc.dram_tensor(n, [128, NHC, QH * D], f8, kind="ExternalInput").ap()
        for n in ("wq8h", "wq8l")
    ]
    wk8_d = [
        nc.dram_tensor(n, [128, NHC, D], f8, kind="ExternalInput").ap()
        for n in ("wk8h", "wk8l")
    ]
    wv8_d = [
        nc.dram_tensor(n, [128, NHC, D], f8, kind="ExternalInput").ap()
        for n in ("wv8h", "wv8l")
    ]
    wo8_d = [
        nc.dram_tensor(n, [128, QH, HID], f8, kind="ExternalInput").ap()
        for n in ("wo8h", "wo8l")
    ]
    cosd = nc.dram_tensor("cosd", [128, NPACK], f16, kind="ExternalInput").ap()
    sind = nc.dram_tensor("sind", [128, NPACK], f16, kind="ExternalInput").ap()
    trid = nc.dram_tensor("trid", [128, NMASK, 128], f16, kind="ExternalInput").ap()
    idnd = nc.dram_tensor("idnd", [128, 128], f16, kind="ExternalInput").ap()
    swpd = nc.dram_tensor("swpd", [128, 128], f16, kind="ExternalInput").ap()
    out = nc.dram_tensor("out", [NT * 128, HID], f16, kind="ExternalOutput").ap()

    with tile.TileContext(nc) as tc, ExitStack() as top:
        persist = top.enter_context(tc.tile_pool(name="persist", bufs=1))
        kT = persist.tile([128, LPAD], f16, tag="kT", name="kT")
        qT = [
            persist.tile([128, LPAD], f16, tag=f"qT{h}", name=f"qT{h}")
            for h in range(QH)
        ]
        vT = persist.tile([128, LPAD], f16, tag="vT", name="vT")
        vA = persist.tile([128, NBLK, 128], f16, tag="vA", name="vA")
        cosT = persist.tile([128, NPACK], f16, tag="cosT", name="cosT")
        sinT = persist.tile([128, NPACK], f16, tag="sinT", name="sinT")
        oh8 = persist.tile([128, QH, NT * 128], f8, tag="oh8", name="oh8")
        ol8 = persist.tile([128, QH, NT * 128], f8, tag="ol8", name="ol8")
        msk = persist.tile([128, NMASK, 128], f16, tag="msk", name="msk")
        ones = persist.tile([128, 1], f16, tag="ones", name="ones")
        swp = persist.tile([128, 128], f16, tag="swp", name="swp")
        idn = persist.tile([128, 128], f16, tag="idn", name="idn")

        wpool = top.enter_context(tc.tile_pool(name="wpool", bufs=1))
        x8h = wpool.tile([128, NHC, NPACK], f8, tag="x8h", name="x8h")
        x8l = wpool.tile([128, NHC, NPACK], f8, tag="x8l", name="x8l")
        wq8 = [
            wpool.tile([128, NHC, QH * D], f8, tag=f"wq8{i}", name=f"wq8{i}")
            for i in range(2)
        ]
        wk8 = [
            wpool.tile([128, NHC, D], f8, tag=f"wk8{i}", name=f"wk8{i}")
            for i in range(2)
        ]
        wv8 = [
            wpool.tile([128, NHC, D], f8, tag=f"wv8{i}", name=f"wv8{i}")
            for i in range(2)
        ]
        wo8 = [
            wpool.tile([128, QH, HID], f8, tag=f"wo8{i}", name=f"wo8{i}")
            for i in range(2)
        ]

        # ---- DMAs: few and fat, spread over four queues ----
        # x pieces grouped by segment pairs so projection of segment s only
        # waits for its own piece.
        xsplits = [nbase[0], nbase[2], nbase[4], nbase[6], nbase[8]]
        for i in range(4):
            c0, c1 = xsplits[i], xsplits[i + 1]
            nc.sync.dma_start(x8h[:, :, c0:c1], x8h_d[:, :, c0:c1])
            nc.scalar.dma_start(x8l[:, :, c0:c1], x8l_d[:, :, c0:c1])
        nc.gpsimd.dma_start(wk8[0][:], wk8_d[0][:])
        nc.gpsimd.dma_start(wk8[1][:], wk8_d[1][:])
        nc.gpsimd.dma_start(swp[:], swpd[:])
        nc.gpsimd.dma_start(cosT[:], cosd[:])
        nc.gpsimd.dma_start(sinT[:], sind[:])
        nc.sync.dma_start(wq8[0][:], wq8_d[0][:])
        nc.scalar.dma_start(wq8[1][:], wq8_d[1][:])
        nc.sync.dma_start(wv8[0][:], wv8_d[0][:])
        nc.scalar.dma_start(wv8[1][:], wv8_d[1][:])
        nc.sync.dma_start(idnd_dma := idn[:], idnd[:])
        nc.scalar.dma_start(msk[:], trid[:])
        nc.sync.dma_start(wo8[0][:], wo8_d[0][:])
        nc.scalar.dma_start(wo8[1][:], wo8_d[1][:])

        nc.any.memset(ones[:], V0)

        # zero the padded tails of kT/qT/vT so stale SBUF never reaches a
        # matmul (NaN bit patterns would poison even masked entries).
        mse = [nc.vector, nc.gpsimd]
        mi = 0
        for s in range(NSEG):
            w = int(nhat[s])
            p0 = pbase[s] * 128 + w
            p1 = pbase[s + 1] * 128
            if p1 > p0:
                for t in (kT, vT, *qT):
                    mse[mi % 2].memset(t[:, p0:p1], 0.0)
                    mi += 1

        tpool = top.enter_context(tc.tile_pool(name="tpool", bufs=3))
        ppool = top.enter_context(tc.tile_pool(name="ppool", bufs=3))
        spool = top.enter_context(tc.tile_pool(name="spool", bufs=3))
        obpool = top.enter_context(tc.tile_pool(name="obpool", bufs=2))
        psP = top.enter_context(tc.tile_pool(name="psP", bufs=2, space="PSUM"))
        psW = top.enter_context(tc.tile_pool(name="psW", bufs=2, space="PSUM"))
        psS = top.enter_context(tc.tile_pool(name="psS", bufs=2, space="PSUM"))
        psO = top.enter_context(tc.tile_pool(name="psO", bufs=2, space="PSUM"))

        # ---- phase 1: projections + rope (packed coords -> padded coords) ----
        def proj_accum(ps, w8, hb0, hb1, c0, c1):
            """ps[:, :W] += W^T x over all 16 k-tiles, fp8 compensated."""
            n = 0
            total = 3 * NHC // 2
            for cp in range(0, NHC, 2):
                for wi, xi in ((0, 0), (0, 1), (1, 0)):
                    lhsT = w8[wi][:, cp : cp + 2, hb0:hb1]
                    rhs = (x8h if xi == 0 else x8l)[:, cp : cp + 2, c0:c1]
                    nc.tensor.matmul(
                        ps,
                        lhsT,
                        rhs,
                        start=(n == 0),
                        stop=(n == total - 1),
                        perf_mode=DR,
                    )
                    n += 1

        # emit order: all projection accumulations for a segment, with each
        # rope swap matmul deferred until after the next hb's projection so
        # PE never waits on the ACT plain-copy.
        pend = []  # deferred swap work: (plain, cols_packed, dst, pcol0, W)
        swctr = [0]

        def flush_swap():
            if not pend:
                return
            plain, c0, c1, dst, p0 = pend.pop(0)
            w = c1 - c0
            u = swctr[0]
            swctr[0] += 1
            sw = psW.tile([128, 512], f32, tag="sw", name=f"sw{u}")
            nc.tensor.matmul(
                sw[:, :w], swp[:], plain[:, :w], start=True, stop=True
            )
            t1 = tpool.tile([128, 512], f16, tag="t1", name=f"t1_{u}")
            nc.gpsimd.tensor_mul(t1[:, :w], plain[:, :w], cosT[:, c0:c1])
            t2 = tpool.tile([128, 512], f16, tag="t2", name=f"t2_{u}")
            nc.vector.tensor_mul(t2[:, :w], sw[:, :w], sinT[:, c0:c1])
            nc.gpsimd.tensor_add(dst[:, p0 : p0 + w], t1[:, :w], t2[:, :w])

        for s in range(NSEG):
            W = int(nhat[s])
            c0, c1 = nbase[s], nbase[s] + W
            p0 = pbase[s] * 128
            # k, v first (small weights arrive first), then q heads
            for hb in ("k", "v", 0, 1, 2, 3):
                ps = psP.tile([128, 512], f32, tag="ps", name=f"ps{s}_{hb}")
                if hb == "k":
                    proj_accum(ps[:, :W], wk8, 0, D, c0, c1)
                elif hb == "v":
                    proj_accum(ps[:, :W], wv8, 0, D, c0, c1)
                else:
                    proj_accum(ps[:, :W], wq8, hb * D, (hb + 1) * D, c0, c1)
                if hb == "v":
                    nc.scalar.copy(vT[:, p0 : p0 + W], ps[:, :W])
                else:
                    plain = tpool.tile(
                        [128, 512], f16, tag="plain", name=f"pl{s}_{hb}"
                    )
                    nc.scalar.copy(plain[:, :W], ps[:, :W])
                    dst = kT if hb == "k" else qT[hb]
                    pend.append((plain, c0, c1, dst, p0))
                    if len(pend) > 1:
                        flush_swap()
            # v transposes for this segment's blocks
            for i in range(T[s]):
                kb = pbase[s] + i
                vt = psW.tile([128, 512], f32, tag="sw", name=f"vt{kb}")
                nc.tensor.matmul(
                    vt[:, :128],
                    vT[:, kb * 128 : (kb + 1) * 128],
                    idn[:],
                    start=True,
                    stop=True,
                )
                nc.scalar.copy(vA[:, kb, :], vt[:, :128])
            flush_swap()
        flush_swap()

        # ---- phase 2: segment-blocked attention (padded coords) ----
        def cp(eng, out_ap, in_ap):
            if eng is nc.scalar:
                eng.copy(out_ap, in_ap)
            else:
                eng.tensor_copy(out_ap, in_ap)

        eng_oh = [nc.scalar, nc.vector]
        eng_ol = [nc.gpsimd, nc.vector]
        nblk_j = []
        for s in range(NSEG):
            rem = int(nhat[s]) - (T[s] - 1) * 128
            for i in range(T[s]):
                nblk_j.append((s, i, 0 if (i < T[s] - 1 or rem == 128) else 1 + s))
        for j, (s, i, midx) in enumerate(nblk_j):
            nkb = i + 1
            jj = pbase[s] + i
            w = min(128, int(nhat[s]) - i * 128)
            nj0 = nbase[s] + i * 128
            for h in range(QH):
                s_ps = psS.tile([128, 4, 128], f32, tag="S", name=f"s{j}_{h}")
                for ib in range(nkb):
                    kb = pbase[s] + ib
                    nc.tensor.matmul(
                        s_ps[:, ib, :],
                        kT[:, kb * 128 : (kb + 1) * 128],
                        qT[h][:, jj * 128 : (jj + 1) * 128],
                        start=True,
                        stop=True,
                    )
                P = ppool.tile([128, 4, 128], f16, tag="P", name=f"p{j}_{h}")
                nc.scalar.activation(
                    P[:, :nkb, :], s_ps[:, :nkb, :], EXP, scale=SCALE_EFF
                )
                nc.vector.tensor_mul(
                    P[:, nkb - 1, :], P[:, nkb - 1, :], msk[:, midx, :]
                )
                l_ps = psO.tile([128, 256], f32, tag="o", name=f"lo{j}_{h}")
                for ib in range(nkb):
                    nc.tensor.matmul(
                        l_ps[0:1, 128:256],
                        ones[:],
                        P[:, ib, :],
                        start=(ib == 0),
                        stop=(ib == nkb - 1),
                    )
                for ib in range(nkb):
                    kb = pbase[s] + ib
                    nc.tensor.matmul(
                        l_ps[:, 0:128],
                        vA[:, kb, :],
                        P[:, ib, :],
                        start=(ib == 0),
                        stop=(ib == nkb - 1),
                    )
                rc = spool.tile([1, 128], f32, tag="rc", name=f"rc{j}_{h}")
                nc.vector.reciprocal(rc[:], l_ps[0:1, 128:256])
                rb = spool.tile([128, 128], f32, tag="rb", name=f"rb{j}_{h}")
                nc.gpsimd.partition_broadcast(rb[:], rc[:])
                t16 = spool.tile([128, 128], f16, tag="t16", name=f"t16{j}_{h}")
                nc.vector.tensor_mul(t16[:, :w], l_ps[:, 0:w], rb[:, :w])
                cp(eng_oh[(j + h) % 2], oh8[:, h, nj0 : nj0 + w], t16[:, :w])
                eng_ol[(j + h) % 2].tensor_sub(
                    ol8[:, h, nj0 : nj0 + w], t16[:, :w], oh8[:, h, nj0 : nj0 + w]
                )

        # ---- phase 3: output projection (fp8 compensated, packed coords) ----
        eng_ob = [nc.scalar, nc.vector]
        for tb in range(NT):
            w = min(128, NPACK - tb * 128)
            t0 = tb * 128
            ob = obpool.tile([128, HID], f16, tag="ob", name=f"ob{tb}")
            for hc in range(HID // 512):
                f_ps = psP.tile([128, 512], f32, tag="ps", name=f"f{tb}_{hc}")
                n = 0
                for oi, wi in ((0, 0), (0, 1), (1, 0)):
                    o8 = oh8 if oi == 0 else ol8
                    w8 = wo8[wi]
                    for hp in (0, 2):
                        nc.tensor.matmul(
                            f_ps[:w, :],
                            o8[:, hp : hp + 2, t0 : t0 + w],
                            w8[:, hp : hp + 2, hc * 512 : (hc + 1) * 512],
                            start=(n == 0),
                            stop=(n == 5),
                            perf_mode=DR,
                        )
                        n += 1
                cp(eng_ob[hc % 2], ob[:w, hc * 512 : (hc + 1) * 512], f_ps[:w, :])
            nc.sync.dma_start(out[t0 : t0 + w, :], ob[:w, :])

    nc.compile()
    return nc


def _get_nc(T, nhat):
    key = (T, nhat)
    if key not in _CACHE:
        _CACHE[key] = _build_nc(T, nhat)
    return _CACHE[key]


def _split8(a):
    import ml_dtypes

    e4 = ml_dtypes.float8_e4m3
    hi = a.astype(e4)
    lo = (a - hi.astype(np.float32)).astype(e4)
    return hi, lo


def kernel(hidden_states, Wq, Wk, Wv, Wo, sid, position_ids):
    global LAST_EXEC_NS, LAST_RUN_WALL_S
    import time

    from concourse.bass_utils import run_bass_kernel_spmd

    hidden = np.asarray(hidden_states, dtype=np.float32)
    Wq = np.asarray(Wq, dtype=np.float32)
    Wk = np.asarray(Wk, dtype=np.float32)
    Wv = np.asarray(Wv, dtype=np.float32)
    Wo = np.asarray(Wo, dtype=np.float32)
    sid = np.asarray(sid)
    position_ids = np.asarray(position_ids)

    T, nhat, perms, counts = _structure(sid)
    nc = _get_nc(T, nhat)

    NBLK = sum(T)
    NPACK = int(sum(nhat))
    NT = (NPACK + 127) // 128
    nbase = np.cumsum([0] + list(nhat)).tolist()
    NMASK = 1 + NSEG

    f16 = np.float16

    # constants shared by all cores
    swpn = np.zeros((128, 128), f16)
    swpn[(np.arange(128) + 64) % 128, np.arange(128)] = 1.0
    idnn = np.eye(128, dtype=f16)
    ki = np.arange(128)[:, None]
    qi = np.arange(128)[None, :]
    tri = (ki <= qi).astype(f16)
    trin = np.zeros((128, NMASK, 128), f16)
    trin[:, 0, :] = tri
    for s in range(NSEG):
        rem = int(nhat[s]) - (T[s] - 1) * 128
        trin[:, 1 + s, :] = tri * (ki < rem)

    # weights per TP group (shared across batches)
    wgrp = []
    for g in range(TP):
        wq_dev = np.ascontiguousarray(
            (SW * Wq[g * 512 : (g + 1) * 512]).T
        ).reshape(NHC, 128, QH * D)
        wk_dev = np.ascontiguousarray(
            (SW * Wk[g * 128 : (g + 1) * 128]).T
        ).reshape(NHC, 128, D)
        wv_dev = np.ascontiguousarray(
            (SW * Wv[g * 128 : (g + 1) * 128]).T
        ).reshape(NHC, 128, D)
        # wo8[p, h, n] = SWO * Wo[n, g*512 + h*128 + p]
        wo_dev = np.ascontiguousarray(
            (SWO * Wo[:, g * 512 : (g + 1) * 512]).T.reshape(QH, 128, HID)
        ).transpose(1, 0, 2)
        ws = {}
        for name, a in (("wq8", wq_dev), ("wk8", wk_dev), ("wv8", wv_dev)):
            hi, lo = _split8(np.ascontiguousarray(a.transpose(1, 0, 2)))
            ws[name + "h"], ws[name + "l"] = hi, lo
        hi, lo = _split8(np.ascontiguousarray(wo_dev))
        ws["wo8h"], ws["wo8l"] = hi, lo
        wgrp.append(ws)

    in_maps = []
    real_rows = []
    for b in range(B):
        perm = perms[b]
        n_b = counts[b]
        # n-hat-packed x with zero fill between n_b and nhat
        xs = hidden[b].T[:, perm]  # [HID, L] sorted
        xpack = np.zeros((HID, NPACK), np.float32)
        pos = np.zeros(NPACK, np.float32)
        rows = []
        off = 0
        for s in range(NSEG):
            w = int(n_b[s])
            xpack[:, nbase[s] : nbase[s] + w] = xs[:, off : off + w] * SX
            pos[nbase[s] : nbase[s] + w] = position_ids[b][
                perm[off : off + w]
            ].astype(np.float32)
            rows.append(nbase[s] + np.arange(w))
            off += w
        real_rows.append(np.concatenate(rows))

        x8h, x8l = _split8(
            np.ascontiguousarray(xpack.reshape(NHC, 128, NPACK).transpose(1, 0, 2))
        )

        inv = 1.0 / (
            THETA ** (np.arange(0, D, 2, dtype=np.float32) / np.float32(D))
        )
        fr = pos[:, None] * inv[None, :]
        emb = np.concatenate([fr, fr], axis=1)  # [NPACK, D]
        cosT = np.ascontiguousarray(np.cos(emb).T.astype(f16))
        sinT = np.sin(emb).T.astype(np.float32).copy()
        sinT[: D // 2] *= -1.0  # fold rotate_half sign
        sinT = np.ascontiguousarray(sinT.astype(f16))

        for g in range(TP):
            m = dict(
                x8h=x8h,
                x8l=x8l,
                cosd=cosT,
                sind=sinT,
                trid=trin,
                idnd=idnn,
                swpd=swpn,
            )
            m.update(wgrp[g])
            in_maps.append(m)

    t0 = time.time()
    res = run_bass_kernel_spmd(nc, in_maps, core_ids=list(range(NCORES)))
    LAST_RUN_WALL_S = time.time() - t0
    LAST_EXEC_NS = res.exec_time_ns

    full = np.empty((B, L, HID), np.float32)
    for b in range(B):
        acc = np.asarray(res.results[4 * b]["out"]).astype(np.float32)
        for g in range(1, TP):
            acc += np.asarray(res.results[4 * b + g]["out"]).astype(np.float32)
        unp = np.empty((L, HID), np.float32)
        unp[perms[b]] = acc[real_rows[b]]
        full[b] = unp * OUT_DESCALE
    return full


# revision 15
# speedup vs baseline: 1.0377x; 1.0133x over previous
"""Self-contained Trainium2 Bass kernel for BoSs (block-of-states) attention.

Strategy (8 NeuronCores):
  - data-parallel over batch (2) x tensor-parallel over heads (4):
    core c handles batch c//4, q-heads [4g:4g+4] and kv-head g where g=c%4.
  - host sorts tokens by state id with states relabeled by descending segment
    length (so both batches produce the same padded block structure), then
    pads each segment to a multiple of 128.  In padded coordinates the BoSs
    mask is exactly: blocks within one segment, causal, with a single shared
    lower-triangle mask on diagonal blocks (plus a per-segment tail mask on
    the segment's last block).  The sliding window (1024) never binds since
    segments are ~280 tokens.
  - projections and the output GEMM run on the n-hat-packed (unpadded) token
    axis so no FLOPs are spent on padding.
  - fp8 (e4m3) DoubleRow matmuls with hi+lo error compensation for the q/k/v
    projections and the Wo GEMM: x = xh+xl, W = Wh+Wl (host-split after
    scaling into e4m3's sweet spot); the three cross terms xh*Wh, xh*Wl,
    xl*Wh are computed with paired-k-tile DoubleRow instructions (2 k-tiles
    per instruction at 0.5 cycles/row) -> 1.33x over fp16 at ~1e-3 accuracy.
  - attention (scores, softmax denominator, AV) stays fp16: its contraction
    depth (128) is too short for the pairing to pay for the extra casts.
  - global scales (inputs *8, weights *512) keep every fp8 split well above
    the e4m3 subnormal floor; the exp() activation scale and a final host
    divide undo them exactly.
"""

import numpy as np
from contextlib import ExitStack

# problem constants (hardcoded per spec)
B, L, HID = 2, 2048, 2048
H, KVH, D = 16, 4, 128
THETA = 10000.0
NCORES = 8
TP = 4            # tensor-parallel group size (cores per batch)
QH = H // TP      # q heads per core = 4
NHC = HID // 128  # 16 hidden-dim chunks
NSEG = 8
SCALE = float(D) ** -0.5

# fp8 scaling: values ~N(0, 8..10) sit mid-range in e4m3 so the hi/lo split
# residuals stay far above the subnormal floor (2^-9).
SX = 8.0
SW = 512.0
SWO = 512.0
V0 = 128.0        # folded into the softmax-denominator ones vector: oT = o/V0
SCALE_EFF = SCALE / (SW * SX) ** 2
OUT_DESCALE = V0 / (SWO * SW * SX)

_CACHE = {}
LAST_EXEC_NS = None
LAST_RUN_WALL_S = None


def _structure(sid):
    """Shared padded block structure from both batches' state histograms."""
    counts = []
    perms = []
    for b in range(B):
        s = np.asarray(sid[b]).astype(np.int64)
        n = np.bincount(s, minlength=NSEG)
        order = np.argsort(-n, kind="stable")       # states by length desc
        rank = np.empty(NSEG, np.int64)
        rank[order] = np.arange(NSEG)
        perm = np.argsort(rank[s], kind="stable")   # tokens by (rank, pos)
        counts.append(np.sort(n)[::-1])
        perms.append(perm)
    nhat = np.maximum(counts[0], counts[1])
    T = np.maximum(1, np.ceil(nhat / 128).astype(np.int64))
    assert nhat.max() <= 512, f"segment too long: {nhat.max()}"
    assert T.max() <= 4
    return tuple(int(t) for t in T), tuple(int(v) for v in nhat), perms, counts


def _build_nc(T, nhat):
    import concourse.tile as tile
    from concourse import bacc, mybir

    f32 = mybir.dt.float32
    f16 = mybir.dt.float16
    f8 = mybir.dt.float8e4
    EXP = mybir.ActivationFunctionType.Exp
    DR = mybir.MatmulPerfMode.DoubleRow

    NBLK = sum(T)
    LPAD = 128 * NBLK
    NPACK = int(sum(nhat))
    NT = (NPACK + 127) // 128          # Wo token tiles
    pbase = np.cumsum([0] + list(T)).tolist()
    nbase = np.cumsum([0] + list(nhat)).tolist()
    NMASK = 1 + NSEG

    nc = bacc.Bacc(
        "TRN2", target_bir_lowering=False, debug=False, num_devices=NCORES
    )

    x8h_d = nc.dram_tensor("x8h", [128, NHC, NPACK], f8, kind="ExternalInput").ap()
    x8l_d = nc.dram_tensor("x8l", [128, NHC, NPACK], f8, kind="ExternalInput").ap()
    wq8_d = [
        nc.dram_tensor(n, [128, NHC, QH * D], f8, kind="ExternalInput").ap()
        for n in ("wq8h", "wq8l")
    ]
    wk8_d = [
        nc.dram_tensor(n, [128, NHC, D], f8, kind="ExternalInput").ap()
        for n in ("wk8h", "wk8l")
    ]
    wv8_d = [
        nc.dram_tensor(n, [128, NHC, D], f8, kind="ExternalInput").ap()
        for n in ("wv8h", "wv8l")
    ]
    wo8_d = [
        nc.dram_tensor(n, [128, QH, HID], f8, kind="ExternalInput").ap()
        for n in ("wo8h", "wo8l")
    ]
    cosd = nc.dram_tensor("cosd", [128, NPACK], f16, kind="ExternalInput").ap()
    sind = nc.dram_tensor("sind", [128, NPACK], f16, kind="ExternalInput").ap()
    trid = nc.dram_tensor("trid", [128, NMASK, 128], f16, kind="ExternalInput").ap()
    idnd = nc.dram_tensor("idnd", [128, 128], f16, kind="ExternalInput").ap()
    swpd = nc.dram_tensor("swpd", [128, 128], f16, kind="ExternalInput").ap()
    out = nc.dram_tensor("out", [NT * 128, HID], f16, kind="ExternalOutput").ap()

    with tile.TileContext(nc) as tc, ExitStack() as top:
        persist = top.enter_context(tc.tile_pool(name="persist", bufs=1))
        kT = persist.tile([128, LPAD], f16, tag="kT", name="kT")
        qT = [
            persist.tile([128, LPAD], f16, tag=f"qT{h}", name=f"qT{h}")
            for h in range(QH)
        ]
        vT = persist.tile([128, LPAD], f16, tag="vT", name="vT")
        vA = persist.tile([128, NBLK, 128], f16, tag="vA", name="vA")
        cosT = persist.tile([128, NPACK], f16, tag="cosT", name="cosT")
        sinT = persist.tile([128, NPACK], f16, tag="sinT", name="sinT")
        oh8 = persist.tile([128, QH, NT * 128], f8, tag="oh8", name="oh8")
        ol8 = persist.tile([128, QH, NT * 128], f8, tag="ol8", name="ol8")
        msk = persist.tile([128, NMASK, 128], f16, tag="msk", name="msk")
        ones = persist.tile([128, 1], f16, tag="ones", name="ones")
        swp = persist.tile([128, 128], f16, tag="swp", name="swp")
        idn = persist.tile([128, 128], f16, tag="idn", name="idn")

        wpool = top.enter_context(tc.tile_pool(name="wpool", bufs=1))
        x8h = wpool.tile([128, NHC, NPACK], f8, tag="x8h", name="x8h")
        x8l = wpool.tile([128, NHC, NPACK], f8, tag="x8l", name="x8l")
        wq8 = [
            wpool.tile([128, NHC, QH * D], f8, tag=f"wq8{i}", name=f"wq8{i}")
            for i in range(2)
        ]
        wk8 = [
            wpool.tile([128, NHC, D], f8, tag=f"wk8{i}", name=f"wk8{i}")
            for i in range(2)
        ]
        wv8 = [
            wpool.tile([128, NHC, D], f8, tag=f"wv8{i}", name=f"wv8{i}")
            for i in range(2)
        ]
        wo8 = [
            wpool.tile([128, QH, HID], f8, tag=f"wo8{i}", name=f"wo8{i}")
            for i in range(2)
        ]

        # ---- DMAs: ordered by first-use; x loaded per segment so the DMA
        # engines (a serial resource in the cost model) serve urgent weights
        # first.
        def xseg(s):
            c0, c1 = nbase[s], nbase[s + 1]
            nc.sync.dma_start(x8h[:, :, c0:c1], x8h_d[:, :, c0:c1])
            nc.scalar.dma_start(x8l[:, :, c0:c1], x8l_d[:, :, c0:c1])

        nc.sync.dma_start(wk8[0][:], wk8_d[0][:])
        nc.scalar.dma_start(wk8[1][:], wk8_d[1][:])
        xseg(0)
        nc.sync.dma_start(wv8[0][:], wv8_d[0][:])
        nc.scalar.dma_start(wv8[1][:], wv8_d[1][:])
        nc.gpsimd.dma_start(swp[:], swpd[:])
        nc.gpsimd.dma_start(cosT[:], cosd[:])
        nc.gpsimd.dma_start(sinT[:], sind[:])
        xseg(1)
        nc.sync.dma_start(wq8[0][:], wq8_d[0][:])
        nc.scalar.dma_start(wq8[1][:], wq8_d[1][:])
        nc.gpsimd.dma_start(idn[:], idnd[:])
        xseg(2)
        nc.gpsimd.dma_start(msk[:], trid[:])
        for s in range(3, NSEG):
            xseg(s)
        nc.sync.dma_start(wo8[0][:], wo8_d[0][:])
        nc.scalar.dma_start(wo8[1][:], wo8_d[1][:])

        nc.any.memset(ones[:], V0)

        # zero the padded tails of kT/qT/vT so stale SBUF never reaches a
        # matmul (NaN bit patterns would poison even masked entries).
        mse = [nc.vector, nc.gpsimd]
        mi = 0
        for s in range(NSEG):
            w = int(nhat[s])
            p0 = pbase[s] * 128 + w
            p1 = pbase[s + 1] * 128
            if p1 > p0:
                for t in (kT, vT, *qT):
                    mse[mi % 2].memset(t[:, p0:p1], 0.0)
                    mi += 1

        tpool = top.enter_context(tc.tile_pool(name="tpool", bufs=3))
        ppool = top.enter_context(tc.tile_pool(name="ppool", bufs=6))
        spool = top.enter_context(tc.tile_pool(name="spool", bufs=5))
        obpool = top.enter_context(tc.tile_pool(name="obpool", bufs=2))
        psP = top.enter_context(tc.tile_pool(name="psP", bufs=2, space="PSUM"))
        psW = top.enter_context(tc.tile_pool(name="psW", bufs=2, space="PSUM"))
        # one bank per (block, head): S in slots 0..nkb-1 (nkb <= 3), the AV
        # output in slot 3, and the softmax denominator in partition 0 of
        # slot 0 (its S region is dead once exp() has run).
        assert max(T) <= 3
        psS = top.enter_context(tc.tile_pool(name="psS", bufs=4, space="PSUM"))

        # ---- phase 1: projections + rope (packed coords -> padded coords) ----
        def proj_accum(ps, w8, hb0, hb1, c0, c1):
            """ps[:, :W] += W^T x over all 16 k-tiles, fp8 compensated."""
            n = 0
            total = 3 * NHC // 2
            for cp in range(0, NHC, 2):
                for wi, xi in ((0, 0), (0, 1), (1, 0)):
                    lhsT = w8[wi][:, cp : cp + 2, hb0:hb1]
                    rhs = (x8h if xi == 0 else x8l)[:, cp : cp + 2, c0:c1]
                    nc.tensor.matmul(
                        ps,
                        lhsT,
                        rhs,
                        start=(n == 0),
                        stop=(n == total - 1),
                        perf_mode=DR,
                    )
                    n += 1

        # emit order: all projection accumulations for a segment, with each
        # rope swap matmul deferred until after the next hb's projection so
        # PE never waits on the ACT plain-copy.
        pend = []  # deferred swap work: (plain, cols_packed, dst, pcol0, W)
        swctr = [0]

        def flush_swap():
            if not pend:
                return
            plain, c0, c1, dst, p0 = pend.pop(0)
            w = c1 - c0
            u = swctr[0]
            swctr[0] += 1
            sw = psW.tile([128, 512], f32, tag="sw", name=f"sw{u}")
            nc.tensor.matmul(
                sw[:, :w], swp[:], plain[:, :w], start=True, stop=True
            )
            t1 = tpool.tile([128, 512], f16, tag="t1", name=f"t1_{u}")
            nc.gpsimd.tensor_mul(t1[:, :w], plain[:, :w], cosT[:, c0:c1])
            t2 = tpool.tile([128, 512], f16, tag="t2", name=f"t2_{u}")
            nc.vector.tensor_mul(t2[:, :w], sw[:, :w], sinT[:, c0:c1])
            nc.gpsimd.tensor_add(dst[:, p0 : p0 + w], t1[:, :w], t2[:, :w])

        for s in range(NSEG):
            W = int(nhat[s])
            c0, c1 = nbase[s], nbase[s] + W
            p0 = pbase[s] * 128
            # k, v first (small weights arrive first), then q heads
            for hb in ("k", "v", 0, 1, 2, 3):
                ps = psP.tile([128, 512], f32, tag="ps", name=f"ps{s}_{hb}")
                if hb == "k":
                    proj_accum(ps[:, :W], wk8, 0, D, c0, c1)
                elif hb == "v":
                    proj_accum(ps[:, :W], wv8, 0, D, c0, c1)
                else:
                    proj_accum(ps[:, :W], wq8, hb * D, (hb + 1) * D, c0, c1)
                if hb == "v":
                    nc.scalar.copy(vT[:, p0 : p0 + W], ps[:, :W])
                else:
                    plain = tpool.tile(
                        [128, 512], f16, tag="plain", name=f"pl{s}_{hb}"
                    )
                    nc.scalar.copy(plain[:, :W], ps[:, :W])
                    dst = kT if hb == "k" else qT[hb]
                    pend.append((plain, c0, c1, dst, p0))
                    if len(pend) > 1:
                        flush_swap()
            # v transposes for this segment's blocks
            for i in range(T[s]):
                kb = pbase[s] + i
                vt = psW.tile([128, 512], f32, tag="sw", name=f"vt{kb}")
                nc.tensor.matmul(
                    vt[:, :128],
                    vT[:, kb * 128 : (kb + 1) * 128],
                    idn[:],
                    start=True,
                    stop=True,
                )
                nc.scalar.copy(vA[:, kb, :], vt[:, :128])
            flush_swap()
        flush_swap()

        # ---- phase 2: segment-blocked attention (padded coords) ----
        def cp(eng, out_ap, in_ap):
            if eng is nc.scalar:
                eng.copy(out_ap, in_ap)
            else:
                eng.tensor_copy(out_ap, in_ap)

        eng_oh = [nc.scalar, nc.vector]
        eng_ol = [nc.gpsimd, nc.vector]
        eng_ob = [nc.scalar, nc.vector]
        wo_next = [0]  # next Wo token-tile to emit

        def emit_wo(ready_cols):
            """Emit Wo tiles whose oh8/ol8 inputs are complete."""
            while wo_next[0] < NT:
                tb = wo_next[0]
                w = min(128, NPACK - tb * 128)
                if tb * 128 + w > ready_cols:
                    return
                t0 = tb * 128
                ob = obpool.tile([128, HID], f16, tag="ob", name=f"ob{tb}")
                for hc in range(HID // 512):
                    fpool = psP if hc % 2 == 0 else psW
                    f_ps = fpool.tile(
                        [128, 512], f32, tag="ps" if hc % 2 == 0 else "sw",
                        name=f"f{tb}_{hc}",
                    )
                    n = 0
                    for oi, wi in ((0, 0), (0, 1), (1, 0)):
                        o8 = oh8 if oi == 0 else ol8
                        w8 = wo8[wi]
                        for hp in (0, 2):
                            nc.tensor.matmul(
                                f_ps[:w, :],
                                o8[:, hp : hp + 2, t0 : t0 + w],
                                w8[:, hp : hp + 2, hc * 512 : (hc + 1) * 512],
                                start=(n == 0),
                                stop=(n == 5),
                                perf_mode=DR,
                            )
                            n += 1
                    cp(eng_ob[hc % 2], ob[:w, hc * 512 : (hc + 1) * 512], f_ps[:w, :])
                nc.sync.dma_start(out[t0 : t0 + w, :], ob[:w, :])
                wo_next[0] += 1

        nblk_j = []
        for s in range(NSEG):
            rem = int(nhat[s]) - (T[s] - 1) * 128
            for i in range(T[s]):
                nblk_j.append((s, i, 0 if (i < T[s] - 1 or rem == 128) else 1 + s))
        for j, (s, i, midx) in enumerate(nblk_j):
            nkb = i + 1
            jj = pbase[s] + i
            w = min(128, int(nhat[s]) - i * 128)
            nj0 = nbase[s] + i * 128
            for h in range(QH):
                s_ps = psS.tile([128, 4, 128], f32, tag="S", name=f"s{j}_{h}")
                for ib in range(nkb):
                    kb = pbase[s] + ib
                    nc.tensor.matmul(
                        s_ps[:, ib, :],
                        kT[:, kb * 128 : (kb + 1) * 128],
                        qT[h][:, jj * 128 : (jj + 1) * 128],
                        start=True,
                        stop=True,
                    )
                P = ppool.tile([128, 4, 128], f16, tag="P", name=f"p{j}_{h}")
                nc.scalar.activation(
                    P[:, :nkb, :], s_ps[:, :nkb, :], EXP, scale=SCALE_EFF
                )
                nc.vector.tensor_mul(
                    P[:, nkb - 1, :], P[:, nkb - 1, :], msk[:, midx, :]
                )
                for ib in range(nkb):
                    nc.tensor.matmul(
                        s_ps[0:1, 0, :],
                        ones[:],
                        P[:, ib, :],
                        start=(ib == 0),
                        stop=(ib == nkb - 1),
                    )
                for ib in range(nkb):
                    kb = pbase[s] + ib
                    nc.tensor.matmul(
                        s_ps[:, 3, :],
                        vA[:, kb, :],
                        P[:, ib, :],
                        start=(ib == 0),
                        stop=(ib == nkb - 1),
                    )
                rc = spool.tile([1, 128], f32, tag="rc", name=f"rc{j}_{h}")
                nc.vector.reciprocal(rc[:], s_ps[0:1, 0, :])
                rb = spool.tile([128, 128], f32, tag="rb", name=f"rb{j}_{h}")
                nc.gpsimd.partition_broadcast(rb[:], rc[:])
                t16 = spool.tile([128, 128], f16, tag="t16", name=f"t16{j}_{h}")
                nc.vector.tensor_mul(t16[:, :w], s_ps[:, 3, 0:w], rb[:, :w])
                cp(eng_oh[(j + h) % 2], oh8[:, h, nj0 : nj0 + w], t16[:, :w])
                eng_ol[(j + h) % 2].tensor_sub(
                    ol8[:, h, nj0 : nj0 + w], t16[:, :w], oh8[:, h, nj0 : nj0 + w]
                )
            emit_wo(nj0 + w)
        emit_wo(NPACK)

    nc.compile()
    return nc


def _get_nc(T, nhat):
    key = (T, nhat)
    if key not in _CACHE:
        _CACHE[key] = _build_nc(T, nhat)
    return _CACHE[key]


def _split8(a):
    import ml_dtypes

    e4 = ml_dtypes.float8_e4m3
    hi = a.astype(e4)
    lo = (a - hi.astype(np.float32)).astype(e4)
    return hi, lo


def kernel(hidden_states, Wq, Wk, Wv, Wo, sid, position_ids):
    global LAST_EXEC_NS, LAST_RUN_WALL_S
    import time

    from concourse.bass_utils import run_bass_kernel_spmd

    hidden = np.asarray(hidden_states, dtype=np.float32)
    Wq = np.asarray(Wq, dtype=np.float32)
    Wk = np.asarray(Wk, dtype=np.float32)
    Wv = np.asarray(Wv, dtype=np.float32)
    Wo = np.asarray(Wo, dtype=np.float32)
    sid = np.asarray(sid)
    position_ids = np.asarray(position_ids)

    T, nhat, perms, counts = _structure(sid)
    nc = _get_nc(T, nhat)

    NBLK = sum(T)
    NPACK = int(sum(nhat))
    NT = (NPACK + 127) // 128
    nbase = np.cumsum([0] + list(nhat)).tolist()
    NMASK = 1 + NSEG

    f16 = np.float16

    # constants shared by all cores
    swpn = np.zeros((128, 128), f16)
    swpn[(np.arange(128) + 64) % 128, np.arange(128)] = 1.0
    idnn = np.eye(128, dtype=f16)
    ki = np.arange(128)[:, None]
    qi = np.arange(128)[None, :]
    tri = (ki <= qi).astype(f16)
    trin = np.zeros((128, NMASK, 128), f16)
    trin[:, 0, :] = tri
    for s in range(NSEG):
        rem = int(nhat[s]) - (T[s] - 1) * 128
        trin[:, 1 + s, :] = tri * (ki < rem)

    # weights per TP group (shared across batches)
    wgrp = []
    for g in range(TP):
        wq_dev = np.ascontiguousarray(
            (SW * Wq[g * 512 : (g + 1) * 512]).T
        ).reshape(NHC, 128, QH * D)
        wk_dev = np.ascontiguousarray(
            (SW * Wk[g * 128 : (g + 1) * 128]).T
        ).reshape(NHC, 128, D)
        wv_dev = np.ascontiguousarray(
            (SW * Wv[g * 128 : (g + 1) * 128]).T
        ).reshape(NHC, 128, D)
        # wo8[p, h, n] = SWO * Wo[n, g*512 + h*128 + p]
        wo_dev = np.ascontiguousarray(
            (SWO * Wo[:, g * 512 : (g + 1) * 512]).T.reshape(QH, 128, HID)
        ).transpose(1, 0, 2)
        ws = {}
        for name, a in (("wq8", wq_dev), ("wk8", wk_dev), ("wv8", wv_dev)):
            hi, lo = _split8(np.ascontiguousarray(a.transpose(1, 0, 2)))
            ws[name + "h"], ws[name + "l"] = hi, lo
        hi, lo = _split8(np.ascontiguousarray(wo_dev))
        ws["wo8h"], ws["wo8l"] = hi, lo
        wgrp.append(ws)

    in_maps = []
    real_rows = []
    for b in range(B):
        perm = perms[b]
        n_b = counts[b]
        # n-hat-packed x with zero fill between n_b and nhat
        xs = hidden[b].T[:, perm]  # [HID, L] sorted
        xpack = np.zeros((HID, NPACK), np.float32)
        pos = np.zeros(NPACK, np.float32)
        rows = []
        off = 0
        for s in range(NSEG):
            w = int(n_b[s])
            xpack[:, nbase[s] : nbase[s] + w] = xs[:, off : off + w] * SX
            pos[nbase[s] : nbase[s] + w] = position_ids[b][
                perm[off : off + w]
            ].astype(np.float32)
            rows.append(nbase[s] + np.arange(w))
            off += w
        real_rows.append(np.concatenate(rows))

        x8h, x8l = _split8(
            np.ascontiguousarray(xpack.reshape(NHC, 128, NPACK).transpose(1, 0, 2))
        )

        inv = 1.0 / (
            THETA ** (np.arange(0, D, 2, dtype=np.float32) / np.float32(D))
        )
        fr = pos[:, None] * inv[None, :]
        emb = np.concatenate([fr, fr], axis=1)  # [NPACK, D]
        cosT = np.ascontiguousarray(np.cos(emb).T.astype(f16))
        sinT = np.sin(emb).T.astype(np.float32).copy()
        sinT[: D // 2] *= -1.0  # fold rotate_half sign
        sinT = np.ascontiguousarray(sinT.astype(f16))

        for g in range(TP):
            m = dict(
                x8h=x8h,
                x8l=x8l,
                cosd=cosT,
                sind=sinT,
                trid=trin,
                idnd=idnn,
                swpd=swpn,
            )
            m.update(wgrp[g])
            in_maps.append(m)

    t0 = time.time()
    res = run_bass_kernel_spmd(nc, in_maps, core_ids=list(range(NCORES)))
    LAST_RUN_WALL_S = time.time() - t0
    LAST_EXEC_NS = res.exec_time_ns

    full = np.empty((B, L, HID), np.float32)
    for b in range(B):
        acc = np.asarray(res.results[4 * b]["out"]).astype(np.float32)
        for g in range(1, TP):
            acc += np.asarray(res.results[4 * b + g]["out"]).astype(np.float32)
        unp = np.empty((L, HID), np.float32)
        unp[perms[b]] = acc[real_rows[b]]
        full[b] = unp * OUT_DESCALE
    return full


# revision 17
# speedup vs baseline: 1.2378x; 1.1928x over previous
"""Self-contained Trainium2 Bass kernel for BoSs (block-of-states) attention.

Strategy (8 NeuronCores):
  - data-parallel over batch (2) x tensor-parallel over heads (4):
    core c handles batch c//4, q-heads [4g:4g+4] and kv-head g where g=c%4.
  - host sorts tokens by state id with states relabeled by descending segment
    length (so both batches produce the same padded block structure), then
    pads each segment to a multiple of 128.  In padded coordinates the BoSs
    mask is exactly: blocks within one segment, causal, with a single shared
    lower-triangle mask on diagonal blocks (plus a per-segment tail mask on
    the segment's last block).  The sliding window (1024) never binds since
    segments are ~280 tokens.
  - projections and the output GEMM run on the n-hat-packed (unpadded) token
    axis so no FLOPs are spent on padding.
  - fp8 (e4m3) DoubleRow matmuls with hi+lo error compensation for the q/k/v
    projections and the Wo GEMM: x = xh+xl, W = Wh+Wl (host-split after
    scaling into e4m3's sweet spot); the three cross terms xh*Wh, xh*Wl,
    xl*Wh are computed with paired-k-tile DoubleRow instructions (2 k-tiles
    per instruction at 0.5 cycles/row) -> 1.33x over fp16 at ~1e-3 accuracy.
  - attention (scores, softmax denominator, AV) stays fp16: its contraction
    depth (128) is too short for the pairing to pay for the extra casts.
  - global scales (inputs *8, weights *512) keep every fp8 split well above
    the e4m3 subnormal floor; the exp() activation scale and a final host
    divide undo them exactly.
"""

import numpy as np
from contextlib import ExitStack

# problem constants (hardcoded per spec)
B, L, HID = 2, 2048, 2048
H, KVH, D = 16, 4, 128
THETA = 10000.0
NCORES = 8
TP = 4            # tensor-parallel group size (cores per batch)
QH = H // TP      # q heads per core = 4
NHC = HID // 128  # 16 hidden-dim chunks
NSEG = 8
SCALE = float(D) ** -0.5

# fp8 scaling: values ~N(0, 8..10) sit mid-range in e4m3 so the hi/lo split
# residuals stay far above the subnormal floor (2^-9).
SX = 8.0
SW = 512.0
SWO = 512.0
V0 = 128.0        # folded into the softmax-denominator ones vector: oT = o/V0
SCALE_EFF = SCALE / (SW * SX) ** 2
OUT_DESCALE = V0 / (SWO * SW * SX)

_CACHE = {}
LAST_EXEC_NS = None
LAST_RUN_WALL_S = None


def _structure(sid):
    """Shared padded block structure from both batches' state histograms."""
    counts = []
    perms = []
    for b in range(B):
        s = np.asarray(sid[b]).astype(np.int64)
        n = np.bincount(s, minlength=NSEG)
        order = np.argsort(-n, kind="stable")       # states by length desc
        rank = np.empty(NSEG, np.int64)
        rank[order] = np.arange(NSEG)
        perm = np.argsort(rank[s], kind="stable")   # tokens by (rank, pos)
        counts.append(np.sort(n)[::-1])
        perms.append(perm)
    nhat = np.maximum(counts[0], counts[1])
    T = np.maximum(1, np.ceil(nhat / 128).astype(np.int64))
    assert nhat.max() <= 512, f"segment too long: {nhat.max()}"
    assert T.max() <= 4
    return tuple(int(t) for t in T), tuple(int(v) for v in nhat), perms, counts


def _build_nc(T, nhat):
    import concourse.tile as tile
    from concourse import bacc, mybir

    f32 = mybir.dt.float32
    f16 = mybir.dt.float16
    f8 = mybir.dt.float8e4
    EXP = mybir.ActivationFunctionType.Exp
    DR = mybir.MatmulPerfMode.DoubleRow

    NBLK = sum(T)
    LPAD = 128 * NBLK
    NPACK = int(sum(nhat))
    NT = (NPACK + 127) // 128          # Wo token tiles
    pbase = np.cumsum([0] + list(T)).tolist()
    nbase = np.cumsum([0] + list(nhat)).tolist()
    NMASK = 1 + NSEG

    nc = bacc.Bacc(
        "TRN2", target_bir_lowering=False, debug=False, num_devices=NCORES
    )

    x8h_d = nc.dram_tensor("x8h", [128, NHC, NPACK], f8, kind="ExternalInput").ap()
    x8l_d = nc.dram_tensor("x8l", [128, NHC, NPACK], f8, kind="ExternalInput").ap()
    wq8_d = [
        nc.dram_tensor(n, [128, NHC, QH * D], f8, kind="ExternalInput").ap()
        for n in ("wq8h", "wq8l")
    ]
    wk8_d = [
        nc.dram_tensor(n, [128, NHC, D], f8, kind="ExternalInput").ap()
        for n in ("wk8h", "wk8l")
    ]
    wv8_d = [
        nc.dram_tensor(n, [128, NHC, D], f8, kind="ExternalInput").ap()
        for n in ("wv8h", "wv8l")
    ]
    wo8_d = [
        nc.dram_tensor(n, [128, QH, HID], f8, kind="ExternalInput").ap()
        for n in ("wo8h", "wo8l")
    ]
    cosd = nc.dram_tensor("cosd", [128, NPACK], f16, kind="ExternalInput").ap()
    sind = nc.dram_tensor("sind", [128, NPACK], f16, kind="ExternalInput").ap()
    trid = nc.dram_tensor("trid", [128, NMASK, 128], f16, kind="ExternalInput").ap()
    idnd = nc.dram_tensor("idnd", [128, 128], f16, kind="ExternalInput").ap()
    swpd = nc.dram_tensor("swpd", [128, 128], f16, kind="ExternalInput").ap()
    out = nc.dram_tensor("out", [NT * 128, HID], f16, kind="ExternalOutput").ap()

    with tile.TileContext(nc) as tc, ExitStack() as top:
        persist = top.enter_context(tc.tile_pool(name="persist", bufs=1))
        kT = persist.tile([128, LPAD], f16, tag="kT", name="kT")
        qT = [
            persist.tile([128, LPAD], f16, tag=f"qT{h}", name=f"qT{h}")
            for h in range(QH)
        ]
        vT = persist.tile([128, LPAD], f16, tag="vT", name="vT")
        vA = persist.tile([128, NBLK, 128], f16, tag="vA", name="vA")
        cosT = persist.tile([128, NPACK], f16, tag="cosT", name="cosT")
        sinT = persist.tile([128, NPACK], f16, tag="sinT", name="sinT")
        oh8 = persist.tile([128, QH, NT * 128], f8, tag="oh8", name="oh8")
        ol8 = persist.tile([128, QH, NT * 128], f8, tag="ol8", name="ol8")
        msk = persist.tile([128, NMASK, 128], f16, tag="msk", name="msk")
        ones = persist.tile([128, 1], f16, tag="ones", name="ones")
        swp = persist.tile([128, 128], f16, tag="swp", name="swp")
        idn = persist.tile([128, 128], f16, tag="idn", name="idn")

        wpool = top.enter_context(tc.tile_pool(name="wpool", bufs=1))
        x8h = wpool.tile([128, NHC, NPACK], f8, tag="x8h", name="x8h")
        x8l = wpool.tile([128, NHC, NPACK], f8, tag="x8l", name="x8l")
        wq8 = [
            wpool.tile([128, NHC, QH * D], f8, tag=f"wq8{i}", name=f"wq8{i}")
            for i in range(2)
        ]
        wk8 = [
            wpool.tile([128, NHC, D], f8, tag=f"wk8{i}", name=f"wk8{i}")
            for i in range(2)
        ]
        wv8 = [
            wpool.tile([128, NHC, D], f8, tag=f"wv8{i}", name=f"wv8{i}")
            for i in range(2)
        ]
        wo8 = [
            wpool.tile([128, QH, HID], f8, tag=f"wo8{i}", name=f"wo8{i}")
            for i in range(2)
        ]

        # ---- DMAs: ordered by first-use; x loaded per segment so the DMA
        # engines (a serial resource in the cost model) serve urgent weights
        # first.
        def xseg(s):
            c0, c1 = nbase[s], nbase[s + 1]
            nc.sync.dma_start(x8h[:, :, c0:c1], x8h_d[:, :, c0:c1])
            nc.sync.dma_start(x8l[:, :, c0:c1], x8l_d[:, :, c0:c1])

        nc.sync.dma_start(wk8[0][:], wk8_d[0][:])
        nc.sync.dma_start(wk8[1][:], wk8_d[1][:])
        xseg(0)
        nc.sync.dma_start(wv8[0][:], wv8_d[0][:])
        nc.sync.dma_start(wv8[1][:], wv8_d[1][:])
        nc.sync.dma_start(swp[:], swpd[:])
        nc.sync.dma_start(cosT[:], cosd[:])
        nc.sync.dma_start(sinT[:], sind[:])
        xseg(1)
        nc.sync.dma_start(wq8[0][:], wq8_d[0][:])
        nc.sync.dma_start(wq8[1][:], wq8_d[1][:])
        nc.sync.dma_start(idn[:], idnd[:])
        xseg(2)
        nc.sync.dma_start(msk[:], trid[:])
        for s in range(3, NSEG):
            xseg(s)
        nc.sync.dma_start(wo8[0][:], wo8_d[0][:])
        nc.sync.dma_start(wo8[1][:], wo8_d[1][:])

        nc.gpsimd.memset(ones[:], V0)

        # zero the padded tails of kT/qT/vT so stale SBUF never reaches a
        # matmul (NaN bit patterns would poison even masked entries).
        mse = [nc.vector, nc.gpsimd]
        mi = 0
        for s in range(NSEG):
            w = int(nhat[s])
            p0 = pbase[s] * 128 + w
            p1 = pbase[s + 1] * 128
            if p1 > p0:
                for t in (kT, vT, *qT):
                    mse[mi % 2].memset(t[:, p0:p1], 0.0)
                    mi += 1

        tpool = top.enter_context(tc.tile_pool(name="tpool", bufs=3))
        ppool = top.enter_context(tc.tile_pool(name="ppool", bufs=6))
        spool = top.enter_context(tc.tile_pool(name="spool", bufs=5))
        obpool = top.enter_context(tc.tile_pool(name="obpool", bufs=2))
        psP = top.enter_context(tc.tile_pool(name="psP", bufs=2, space="PSUM"))
        psW = top.enter_context(tc.tile_pool(name="psW", bufs=2, space="PSUM"))
        # one bank per (block, head): S in slots 0..nkb-1 (nkb <= 3), the AV
        # output in slot 3, and the softmax denominator in partition 0 of
        # slot 0 (its S region is dead once exp() has run).
        assert max(T) <= 3
        psS = top.enter_context(tc.tile_pool(name="psS", bufs=4, space="PSUM"))

        # ---- phase 1: projections + rope (packed coords -> padded coords) ----
        def proj_accum(ps, w8, hb0, hb1, c0, c1):
            """ps[:, :W] += W^T x over all 16 k-tiles, fp8 compensated."""
            n = 0
            total = 3 * NHC // 2
            for cp in range(0, NHC, 2):
                for wi, xi in ((0, 0), (0, 1), (1, 0)):
                    lhsT = w8[wi][:, cp : cp + 2, hb0:hb1]
                    rhs = (x8h if xi == 0 else x8l)[:, cp : cp + 2, c0:c1]
                    nc.tensor.matmul(
                        ps,
                        lhsT,
                        rhs,
                        start=(n == 0),
                        stop=(n == total - 1),
                        perf_mode=DR,
                    )
                    n += 1

        # emit order: all projection accumulations for a segment, with each
        # rope swap matmul deferred until after the next hb's projection so
        # PE never waits on the ACT plain-copy.
        pend = []  # deferred swap work: (plain, cols_packed, dst, pcol0, W)
        swctr = [0]

        def flush_swap():
            if not pend:
                return
            plain, c0, c1, dst, p0 = pend.pop(0)
            w = c1 - c0
            u = swctr[0]
            swctr[0] += 1
            sw = psW.tile([128, 512], f32, tag="sw", name=f"sw{u}")
            nc.tensor.matmul(
                sw[:, :w], swp[:], plain[:, :w], start=True, stop=True
            )
            t1 = tpool.tile([128, 512], f16, tag="t1", name=f"t1_{u}")
            nc.gpsimd.tensor_mul(t1[:, :w], plain[:, :w], cosT[:, c0:c1])
            t2 = tpool.tile([128, 512], f16, tag="t2", name=f"t2_{u}")
            nc.vector.tensor_mul(t2[:, :w], sw[:, :w], sinT[:, c0:c1])
            nc.gpsimd.tensor_add(dst[:, p0 : p0 + w], t1[:, :w], t2[:, :w])

        for s in range(NSEG):
            W = int(nhat[s])
            c0, c1 = nbase[s], nbase[s] + W
            p0 = pbase[s] * 128
            # k, v first (small weights arrive first), then q heads
            for hb in ("k", "v", 0, 1, 2, 3):
                ps = psP.tile([128, 512], f32, tag="ps", name=f"ps{s}_{hb}")
                if hb == "k":
                    proj_accum(ps[:, :W], wk8, 0, D, c0, c1)
                elif hb == "v":
                    proj_accum(ps[:, :W], wv8, 0, D, c0, c1)
                else:
                    proj_accum(ps[:, :W], wq8, hb * D, (hb + 1) * D, c0, c1)
                if hb == "v":
                    nc.scalar.copy(vT[:, p0 : p0 + W], ps[:, :W])
                else:
                    plain = tpool.tile(
                        [128, 512], f16, tag="plain", name=f"pl{s}_{hb}"
                    )
                    nc.scalar.copy(plain[:, :W], ps[:, :W])
                    dst = kT if hb == "k" else qT[hb]
                    pend.append((plain, c0, c1, dst, p0))
                    if len(pend) > 1:
                        flush_swap()
            # v transposes for this segment's blocks
            for i in range(T[s]):
                kb = pbase[s] + i
                vt = psW.tile([128, 512], f32, tag="sw", name=f"vt{kb}")
                nc.tensor.matmul(
                    vt[:, :128],
                    vT[:, kb * 128 : (kb + 1) * 128],
                    idn[:],
                    start=True,
                    stop=True,
                )
                nc.scalar.copy(vA[:, kb, :], vt[:, :128])
            flush_swap()
        flush_swap()

        # ---- phase 2: segment-blocked attention (padded coords) ----
        def cp(eng, out_ap, in_ap):
            if eng is nc.scalar:
                eng.copy(out_ap, in_ap)
            else:
                eng.tensor_copy(out_ap, in_ap)

        eng_oh = [nc.scalar, nc.vector]
        eng_ol = [nc.gpsimd, nc.vector]
        eng_ob = [nc.scalar, nc.vector]
        wo_next = [0]  # next Wo token-tile to emit

        def emit_wo(ready_cols):
            """Emit Wo tiles whose oh8/ol8 inputs are complete."""
            while wo_next[0] < NT:
                tb = wo_next[0]
                w = min(128, NPACK - tb * 128)
                if tb * 128 + w > ready_cols:
                    return
                t0 = tb * 128
                ob = obpool.tile([128, HID], f16, tag="ob", name=f"ob{tb}")
                for hc in range(HID // 512):
                    fpool = psP if hc % 2 == 0 else psW
                    f_ps = fpool.tile(
                        [128, 512], f32, tag="ps" if hc % 2 == 0 else "sw",
                        name=f"f{tb}_{hc}",
                    )
                    n = 0
                    for oi, wi in ((0, 0), (0, 1), (1, 0)):
                        o8 = oh8 if oi == 0 else ol8
                        w8 = wo8[wi]
                        for hp in (0, 2):
                            nc.tensor.matmul(
                                f_ps[:w, :],
                                o8[:, hp : hp + 2, t0 : t0 + w],
                                w8[:, hp : hp + 2, hc * 512 : (hc + 1) * 512],
                                start=(n == 0),
                                stop=(n == 5),
                                perf_mode=DR,
                            )
                            n += 1
                    cp(eng_ob[hc % 2], ob[:w, hc * 512 : (hc + 1) * 512], f_ps[:w, :])
                nc.sync.dma_start(out[t0 : t0 + w, :], ob[:w, :])
                wo_next[0] += 1

        nblk_j = []
        for s in range(NSEG):
            rem = int(nhat[s]) - (T[s] - 1) * 128
            for i in range(T[s]):
                nblk_j.append((s, i, 0 if (i < T[s] - 1 or rem == 128) else 1 + s))

        work = [(j, h) for j in range(len(nblk_j)) for h in range(QH)]
        state = {}

        def stage_a(idx):
            j, h = work[idx]
            s, i, midx = nblk_j[j]
            nkb = i + 1
            jj = pbase[s] + i
            s_ps = psS.tile([128, 4, 128], f32, tag="S", name=f"s{j}_{h}")
            for ib in range(nkb):
                kb = pbase[s] + ib
                nc.tensor.matmul(
                    s_ps[:, ib, :],
                    kT[:, kb * 128 : (kb + 1) * 128],
                    qT[h][:, jj * 128 : (jj + 1) * 128],
                    start=True,
                    stop=True,
                )
            P = ppool.tile([128, 4, 128], f16, tag="P", name=f"p{j}_{h}")
            nc.scalar.activation(
                P[:, :nkb, :], s_ps[:, :nkb, :], EXP, scale=SCALE_EFF
            )
            nc.vector.tensor_mul(
                P[:, nkb - 1, :], P[:, nkb - 1, :], msk[:, midx, :]
            )
            state[idx] = (s_ps, P)

        def stage_b(idx):
            j, h = work[idx]
            s, i, midx = nblk_j[j]
            nkb = i + 1
            w = min(128, int(nhat[s]) - i * 128)
            nj0 = nbase[s] + i * 128
            s_ps, P = state.pop(idx)
            for ib in range(nkb):
                nc.tensor.matmul(
                    s_ps[0:1, 0, :],
                    ones[:],
                    P[:, ib, :],
                    start=(ib == 0),
                    stop=(ib == nkb - 1),
                )
            for ib in range(nkb):
                kb = pbase[s] + ib
                nc.tensor.matmul(
                    s_ps[:, 3, :],
                    vA[:, kb, :],
                    P[:, ib, :],
                    start=(ib == 0),
                    stop=(ib == nkb - 1),
                )
            rc = spool.tile([1, 128], f32, tag="rc", name=f"rc{j}_{h}")
            nc.vector.reciprocal(rc[:], s_ps[0:1, 0, :])
            rb = spool.tile([128, 128], f32, tag="rb", name=f"rb{j}_{h}")
            nc.gpsimd.partition_broadcast(rb[:], rc[:])
            t16 = spool.tile([128, 128], f16, tag="t16", name=f"t16{j}_{h}")
            nc.vector.tensor_mul(t16[:, :w], s_ps[:, 3, 0:w], rb[:, :w])
            cp(eng_oh[(j + h) % 2], oh8[:, h, nj0 : nj0 + w], t16[:, :w])
            eng_ol[(j + h) % 2].tensor_sub(
                ol8[:, h, nj0 : nj0 + w], t16[:, :w], oh8[:, h, nj0 : nj0 + w]
            )
            if h == QH - 1 and j >= 1:
                # Wo tiles fully covered by the PREVIOUS block's columns (so
                # the fp8 o-splits they read are long since written)
                sP, iP, _ = nblk_j[j - 1]
                nc_prev = nbase[sP] + iP * 128 + min(
                    128, int(nhat[sP]) - iP * 128
                )
                emit_wo(nc_prev)

        LAG = 2
        for idx in range(len(work)):
            stage_a(idx)
            if idx >= LAG:
                stage_b(idx - LAG)
        for idx in range(len(work) - LAG, len(work)):
            stage_b(idx)
        emit_wo(NPACK)

    nc.compile()
    return nc


def _get_nc(T, nhat):
    key = (T, nhat)
    if key not in _CACHE:
        _CACHE[key] = _build_nc(T, nhat)
    return _CACHE[key]


def _split8(a):
    import ml_dtypes

    e4 = ml_dtypes.float8_e4m3
    hi = a.astype(e4)
    lo = (a - hi.astype(np.float32)).astype(e4)
    return hi, lo


def kernel(hidden_states, Wq, Wk, Wv, Wo, sid, position_ids):
    global LAST_EXEC_NS, LAST_RUN_WALL_S
    import time

    from concourse.bass_utils import run_bass_kernel_spmd

    hidden = np.asarray(hidden_states, dtype=np.float32)
    Wq = np.asarray(Wq, dtype=np.float32)
    Wk = np.asarray(Wk, dtype=np.float32)
    Wv = np.asarray(Wv, dtype=np.float32)
    Wo = np.asarray(Wo, dtype=np.float32)
    sid = np.asarray(sid)
    position_ids = np.asarray(position_ids)

    T, nhat, perms, counts = _structure(sid)
    nc = _get_nc(T, nhat)

    NBLK = sum(T)
    NPACK = int(sum(nhat))
    NT = (NPACK + 127) // 128
    nbase = np.cumsum([0] + list(nhat)).tolist()
    NMASK = 1 + NSEG

    f16 = np.float16

    # constants shared by all cores
    swpn = np.zeros((128, 128), f16)
    swpn[(np.arange(128) + 64) % 128, np.arange(128)] = 1.0
    idnn = np.eye(128, dtype=f16)
    ki = np.arange(128)[:, None]
    qi = np.arange(128)[None, :]
    tri = (ki <= qi).astype(f16)
    trin = np.zeros((128, NMASK, 128), f16)
    trin[:, 0, :] = tri
    for s in range(NSEG):
        rem = int(nhat[s]) - (T[s] - 1) * 128
        trin[:, 1 + s, :] = tri * (ki < rem)

    # weights per TP group (shared across batches)
    wgrp = []
    for g in range(TP):
        wq_dev = np.ascontiguousarray(
            (SW * Wq[g * 512 : (g + 1) * 512]).T
        ).reshape(NHC, 128, QH * D)
        wk_dev = np.ascontiguousarray(
            (SW * Wk[g * 128 : (g + 1) * 128]).T
        ).reshape(NHC, 128, D)
        wv_dev = np.ascontiguousarray(
            (SW * Wv[g * 128 : (g + 1) * 128]).T
        ).reshape(NHC, 128, D)
        # wo8[p, h, n] = SWO * Wo[n, g*512 + h*128 + p]
        wo_dev = np.ascontiguousarray(
            (SWO * Wo[:, g * 512 : (g + 1) * 512]).T.reshape(QH, 128, HID)
        ).transpose(1, 0, 2)
        ws = {}
        for name, a in (("wq8", wq_dev), ("wk8", wk_dev), ("wv8", wv_dev)):
            hi, lo = _split8(np.ascontiguousarray(a.transpose(1, 0, 2)))
            ws[name + "h"], ws[name + "l"] = hi, lo
        hi, lo = _split8(np.ascontiguousarray(wo_dev))
        ws["wo8h"], ws["wo8l"] = hi, lo
        wgrp.append(ws)

    in_maps = []
    real_rows = []
    for b in range(B):
        perm = perms[b]
        n_b = counts[b]
        # n-hat-packed x with zero fill between n_b and nhat
        xs = hidden[b].T[:, perm]  # [HID, L] sorted
        xpack = np.zeros((HID, NPACK), np.float32)
        pos = np.zeros(NPACK, np.float32)
        rows = []
        off = 0
        for s in range(NSEG):
            w = int(n_b[s])
            xpack[:, nbase[s] : nbase[s] + w] = xs[:, off : off + w] * SX
            pos[nbase[s] : nbase[s] + w] = position_ids[b][
                perm[off : off + w]
            ].astype(np.float32)
            rows.append(nbase[s] + np.arange(w))
            off += w
        real_rows.append(np.concatenate(rows))

        x8h, x8l = _split8(
            np.ascontiguousarray(xpack.reshape(NHC, 128, NPACK).transpose(1, 0, 2))
        )

        inv = 1.0 / (
            THETA ** (np.arange(0, D, 2, dtype=np.float32) / np.float32(D))
        )
        fr = pos[:, None] * inv[None, :]
        emb = np.concatenate([fr, fr], axis=1)  # [NPACK, D]
        cosT = np.ascontiguousarray(np.cos(emb).T.astype(f16))
        sinT = np.sin(emb).T.astype(np.float32).copy()
        sinT[: D // 2] *= -1.0  # fold rotate_half sign
        sinT = np.ascontiguousarray(sinT.astype(f16))

        for g in range(TP):
            m = dict(
                x8h=x8h,
                x8l=x8l,
                cosd=cosT,
                sind=sinT,
                trid=trin,
                idnd=idnn,
                swpd=swpn,
            )
            m.update(wgrp[g])
            in_maps.append(m)

    t0 = time.time()
    res = run_bass_kernel_spmd(nc, in_maps, core_ids=list(range(NCORES)))
    LAST_RUN_WALL_S = time.time() - t0
    LAST_EXEC_NS = res.exec_time_ns

    full = np.empty((B, L, HID), np.float32)
    for b in range(B):
        acc = np.asarray(res.results[4 * b]["out"]).astype(np.float32)
        for g in range(1, TP):
            acc += np.asarray(res.results[4 * b + g]["out"]).astype(np.float32)
        unp = np.empty((L, HID), np.float32)
        unp[perms[b]] = acc[real_rows[b]]
        full[b] = unp * OUT_DESCALE
    return full


# revision 26
# speedup vs baseline: 1.2970x; 1.0478x over previous
"""Self-contained Trainium2 Bass kernel for BoSs (block-of-states) attention.

Strategy (8 NeuronCores):
  - data-parallel over batch (2) x tensor-parallel over heads (4):
    core c handles batch c//4, q-heads [4g:4g+4] and kv-head g where g=c%4.
  - host sorts tokens by state id with states relabeled by descending segment
    length (so both batches produce the same padded block structure), then
    pads each segment to a multiple of 128.  In padded coordinates the BoSs
    mask is exactly: blocks within one segment, causal, with a single shared
    lower-triangle mask on diagonal blocks (plus a per-segment tail mask on
    the segment's last block).  The sliding window (1024) never binds since
    segments are ~280 tokens.
  - projections and the output GEMM run on the n-hat-packed (unpadded) token
    axis so no FLOPs are spent on padding.
  - fp8 (e4m3) DoubleRow matmuls with hi+lo error compensation for the q/k/v
    projections and the Wo GEMM: x = xh+xl, W = Wh+Wl (host-split after
    scaling into e4m3's sweet spot); the three cross terms xh*Wh, xh*Wl,
    xl*Wh are computed with paired-k-tile DoubleRow instructions (2 k-tiles
    per instruction at 0.5 cycles/row) -> 1.33x over fp16 at ~1e-3 accuracy.
  - attention (scores, softmax denominator, AV) stays fp16: its contraction
    depth (128) is too short for the pairing to pay for the extra casts.
  - global scales (inputs *8, weights *512) keep every fp8 split well above
    the e4m3 subnormal floor; the exp() activation scale and a final host
    divide undo them exactly.
"""

import numpy as np
from contextlib import ExitStack

# problem constants (hardcoded per spec)
B, L, HID = 2, 2048, 2048
H, KVH, D = 16, 4, 128
THETA = 10000.0
NCORES = 8
TP = 4            # tensor-parallel group size (cores per batch)
QH = H // TP      # q heads per core = 4
NHC = HID // 128  # 16 hidden-dim chunks
NSEG = 8
SCALE = float(D) ** -0.5

# fp8 scaling: values ~N(0, 8..10) sit mid-range in e4m3 so the hi/lo split
# residuals stay far above the subnormal floor (2^-9).
SX = 8.0
SW = 512.0
SWO = 512.0
V0 = 128.0        # folded into the softmax-denominator ones vector: oT = o/V0
SCALE_EFF = SCALE / (SW * SX) ** 2
OUT_DESCALE = V0 / (SWO * SW * SX)

_CACHE = {}
LAST_EXEC_NS = None
LAST_RUN_WALL_S = None


def _structure(sid):
    """Shared padded block structure from both batches' state histograms."""
    counts = []
    perms = []
    for b in range(B):
        s = np.asarray(sid[b]).astype(np.int64)
        n = np.bincount(s, minlength=NSEG)
        order = np.argsort(-n, kind="stable")       # states by length desc
        rank = np.empty(NSEG, np.int64)
        rank[order] = np.arange(NSEG)
        perm = np.argsort(rank[s], kind="stable")   # tokens by (rank, pos)
        counts.append(np.sort(n)[::-1])
        perms.append(perm)
    nhat = np.maximum(counts[0], counts[1])
    T = np.maximum(1, np.ceil(nhat / 128).astype(np.int64))
    assert nhat.max() <= 512, f"segment too long: {nhat.max()}"
    assert T.max() <= 4
    return tuple(int(t) for t in T), tuple(int(v) for v in nhat), perms, counts


def _build_nc(T, nhat):
    import concourse.tile as tile
    from concourse import bacc, mybir

    f32 = mybir.dt.float32
    f16 = mybir.dt.float16
    f8 = mybir.dt.float8e4
    EXP = mybir.ActivationFunctionType.Exp
    DR = mybir.MatmulPerfMode.DoubleRow

    NBLK = sum(T)
    LPAD = 128 * NBLK
    NPACK = int(sum(nhat))
    NT = (NPACK + 127) // 128          # Wo token tiles
    pbase = np.cumsum([0] + list(T)).tolist()
    nbase = np.cumsum([0] + list(nhat)).tolist()
    NMASK = 1 + NSEG

    nc = bacc.Bacc(
        "TRN2", target_bir_lowering=False, debug=False, num_devices=NCORES
    )

    x8h_d = nc.dram_tensor("x8h", [128, NHC, NPACK], f8, kind="ExternalInput").ap()
    x8l_d = nc.dram_tensor("x8l", [128, NHC, NPACK], f8, kind="ExternalInput").ap()
    wq8_d = [
        nc.dram_tensor(n, [128, NHC, QH * D], f8, kind="ExternalInput").ap()
        for n in ("wq8h", "wq8l")
    ]
    wk8_d = [
        nc.dram_tensor(n, [128, NHC, D], f8, kind="ExternalInput").ap()
        for n in ("wk8h", "wk8l")
    ]
    wv8_d = [
        nc.dram_tensor(n, [128, NHC, D], f8, kind="ExternalInput").ap()
        for n in ("wv8h", "wv8l")
    ]
    wo8_d = [
        nc.dram_tensor(n, [128, QH, HID], f8, kind="ExternalInput").ap()
        for n in ("wo8h", "wo8l")
    ]
    cosd = nc.dram_tensor("cosd", [128, NPACK], f16, kind="ExternalInput").ap()
    sind = nc.dram_tensor("sind", [128, NPACK], f16, kind="ExternalInput").ap()
    trid = nc.dram_tensor("trid", [128, NMASK, 128], f16, kind="ExternalInput").ap()
    idnd = nc.dram_tensor("idnd", [128, 128], f16, kind="ExternalInput").ap()
    swpd = nc.dram_tensor("swpd", [128, 128], f16, kind="ExternalInput").ap()
    out = nc.dram_tensor("out", [NT * 128, HID], f16, kind="ExternalOutput").ap()

    with tile.TileContext(nc) as tc, ExitStack() as top:
        persist = top.enter_context(tc.tile_pool(name="persist", bufs=1))
        kT = persist.tile([128, LPAD], f16, tag="kT", name="kT")
        qT = [
            persist.tile([128, LPAD], f16, tag=f"qT{h}", name=f"qT{h}")
            for h in range(QH)
        ]
        vT = persist.tile([128, LPAD], f16, tag="vT", name="vT")
        vA = persist.tile([128, NBLK, 128], f16, tag="vA", name="vA")
        cosT = persist.tile([128, NPACK], f16, tag="cosT", name="cosT")
        sinT = persist.tile([128, NPACK], f16, tag="sinT", name="sinT")
        oh8 = persist.tile([128, QH, NT * 128], f8, tag="oh8", name="oh8")
        ol8 = persist.tile([128, QH, NT * 128], f8, tag="ol8", name="ol8")
        msk = persist.tile([128, NMASK, 128], f16, tag="msk", name="msk")
        ones = persist.tile([128, 1], f16, tag="ones", name="ones")
        ones1 = persist.tile([1, 128], f16, tag="ones1", name="ones1")
        swp = persist.tile([128, 128], f16, tag="swp", name="swp")
        idn = persist.tile([128, 128], f16, tag="idn", name="idn")

        wpool = top.enter_context(tc.tile_pool(name="wpool", bufs=1))
        x8h = wpool.tile([128, NHC, NPACK], f8, tag="x8h", name="x8h")
        x8l = wpool.tile([128, NHC, NPACK], f8, tag="x8l", name="x8l")
        wq8 = [
            wpool.tile([128, NHC, QH * D], f8, tag=f"wq8{i}", name=f"wq8{i}")
            for i in range(2)
        ]
        wk8 = [
            wpool.tile([128, NHC, D], f8, tag=f"wk8{i}", name=f"wk8{i}")
            for i in range(2)
        ]
        wv8 = [
            wpool.tile([128, NHC, D], f8, tag=f"wv8{i}", name=f"wv8{i}")
            for i in range(2)
        ]
        wo8 = [
            wpool.tile([128, QH, HID], f8, tag=f"wo8{i}", name=f"wo8{i}")
            for i in range(2)
        ]

        # ---- DMAs: ordered by first-use; x loaded per segment so the DMA
        # engines (a serial resource in the cost model) serve urgent weights
        # first.
        def xseg(s):
            c0, c1 = nbase[s], nbase[s + 1]
            nc.sync.dma_start(x8h[:, :, c0:c1], x8h_d[:, :, c0:c1])
            nc.sync.dma_start(x8l[:, :, c0:c1], x8l_d[:, :, c0:c1])

        nc.sync.dma_start(wk8[0][:], wk8_d[0][:])
        nc.sync.dma_start(x8h[:, :, nbase[0] : nbase[1]], x8h_d[:, :, nbase[0] : nbase[1]])
        nc.sync.dma_start(wk8[1][:], wk8_d[1][:])
        nc.sync.dma_start(wv8[0][:], wv8_d[0][:])
        nc.sync.dma_start(x8l[:, :, nbase[0] : nbase[1]], x8l_d[:, :, nbase[0] : nbase[1]])
        nc.sync.dma_start(wv8[1][:], wv8_d[1][:])
        nc.sync.dma_start(wq8[0][:], wq8_d[0][:])
        nc.sync.dma_start(swp[:], swpd[:])
        nc.sync.dma_start(cosT[:], cosd[:])
        nc.sync.dma_start(sinT[:], sind[:])
        nc.sync.dma_start(wq8[1][:], wq8_d[1][:])
        xseg(1)
        nc.sync.dma_start(idn[:], idnd[:])
        xseg(2)
        nc.sync.dma_start(msk[:], trid[:])
        for s in range(3, NSEG):
            xseg(s)
        nc.sync.dma_start(wo8[0][:], wo8_d[0][:])
        nc.sync.dma_start(wo8[1][:], wo8_d[1][:])

        # ones=1 feeds the denominator sum; ones1=1/V0 folds o's fp8 range
        # scaling into the 1/l broadcast outer-product.
        nc.gpsimd.memset(ones[:], 1.0)
        nc.gpsimd.memset(ones1[:], 1.0 / V0)

        # zero the padded tails of kT/qT/vT so stale SBUF never reaches a
        # matmul (NaN bit patterns would poison even masked entries).
        mse = [nc.vector, nc.gpsimd]
        mi = 0
        for s in range(NSEG):
            w = int(nhat[s])
            p0 = pbase[s] * 128 + w
            p1 = pbase[s + 1] * 128
            if p1 > p0:
                for t in (kT, vT, *qT):
                    mse[mi % 2].memset(t[:, p0:p1], 0.0)
                    mi += 1

        tpool = top.enter_context(tc.tile_pool(name="tpool", bufs=3))
        ppool = top.enter_context(tc.tile_pool(name="ppool", bufs=6))
        spool = top.enter_context(tc.tile_pool(name="spool", bufs=5))
        obpool = top.enter_context(tc.tile_pool(name="obpool", bufs=2))
        # PSUM (8 banks): psP 2x[128,512] (proj + Wo), psW 1x[128,512] (rope
        # swap), psS 2x[128,3,128] (scores only, short-lived), psLO
        # 3x[128,256] (denominator at [0:1,128:256] -> 1/l broadcast at
        # [:,128:256] after the reciprocal is read, AV output at [:,0:128];
        # also the v-transpose staging in phase 1).
        assert max(T) <= 3
        psP = top.enter_context(tc.tile_pool(name="psP", bufs=2, space="PSUM"))
        psW = top.enter_context(tc.tile_pool(name="psW", bufs=1, space="PSUM"))
        psS = top.enter_context(tc.tile_pool(name="psS", bufs=2, space="PSUM"))
        psLO = top.enter_context(tc.tile_pool(name="psLO", bufs=3, space="PSUM"))

        # ---- phase 1: projections + rope (packed coords -> padded coords) ----
        def proj_accum(ps, w8, hb0, hb1, c0, c1):
            """ps[:, :W] += W^T x over all 16 k-tiles, fp8 compensated.
            Term order (hi*hi, lo*hi, hi*lo) delays the need for the lo
            tensors so their DMAs can trail the hi ones."""
            n = 0
            total = 3 * NHC // 2
            for wi, xi in ((0, 0), (1, 0), (0, 1)):
                for cp in range(0, NHC, 2):
                    lhsT = w8[wi][:, cp : cp + 2, hb0:hb1]
                    rhs = (x8h if xi == 0 else x8l)[:, cp : cp + 2, c0:c1]
                    nc.tensor.matmul(
                        ps,
                        lhsT,
                        rhs,
                        start=(n == 0),
                        stop=(n == total - 1),
                        perf_mode=DR,
                    )
                    n += 1

        # emit order: all projection accumulations for a segment, with each
        # rope swap matmul deferred until after the next hb's projection so
        # PE never waits on the ACT plain-copy.
        pend = []  # deferred swap work: (plain, cols_packed, dst, pcol0, W)
        swctr = [0]

        def flush_swap():
            if not pend:
                return
            plain, c0, c1, dst, p0 = pend.pop(0)
            w = c1 - c0
            u = swctr[0]
            swctr[0] += 1
            sw = psW.tile([128, 512], f32, tag="sw", name=f"sw{u}")
            nc.tensor.matmul(
                sw[:, :w], swp[:], plain[:, :w], start=True, stop=True
            )
            t1 = tpool.tile([128, 512], f16, tag="t1", name=f"t1_{u}")
            nc.gpsimd.tensor_mul(t1[:, :w], plain[:, :w], cosT[:, c0:c1])
            t2 = tpool.tile([128, 512], f16, tag="t2", name=f"t2_{u}")
            nc.vector.tensor_mul(t2[:, :w], sw[:, :w], sinT[:, c0:c1])
            nc.gpsimd.tensor_add(dst[:, p0 : p0 + w], t1[:, :w], t2[:, :w])

        for s in range(NSEG):
            W = int(nhat[s])
            c0, c1 = nbase[s], nbase[s] + W
            p0 = pbase[s] * 128
            # k, v first (small weights arrive first), then q heads
            for hb in ("k", "v", 0, 1, 2, 3):
                ps = psP.tile([128, 512], f32, tag="ps", name=f"ps{s}_{hb}")
                if hb == "k":
                    proj_accum(ps[:, :W], wk8, 0, D, c0, c1)
                elif hb == "v":
                    proj_accum(ps[:, :W], wv8, 0, D, c0, c1)
                else:
                    proj_accum(ps[:, :W], wq8, hb * D, (hb + 1) * D, c0, c1)
                if hb == "v":
                    nc.scalar.copy(vT[:, p0 : p0 + W], ps[:, :W])
                else:
                    plain = tpool.tile(
                        [128, 512], f16, tag="plain", name=f"pl{s}_{hb}"
                    )
                    nc.scalar.copy(plain[:, :W], ps[:, :W])
                    dst = kT if hb == "k" else qT[hb]
                    pend.append((plain, c0, c1, dst, p0))
                    if len(pend) > 1:
                        flush_swap()
            # v transposes for this segment's blocks
            for i in range(T[s]):
                kb = pbase[s] + i
                vt = psLO.tile([128, 256], f32, tag="lo", name=f"vt{kb}")
                nc.tensor.matmul(
                    vt[:, :128],
                    vT[:, kb * 128 : (kb + 1) * 128],
                    idn[:],
                    start=True,
                    stop=True,
                )
                nc.scalar.copy(vA[:, kb, :], vt[:, :128])
            flush_swap()
        flush_swap()

        # ---- phase 2: segment-blocked attention (padded coords) ----
        def cp(eng, out_ap, in_ap):
            if eng is nc.scalar:
                eng.copy(out_ap, in_ap)
            else:
                eng.tensor_copy(out_ap, in_ap)

        eng_oh = [nc.scalar, nc.vector]
        eng_ol = [nc.gpsimd, nc.vector]
        eng_ob = [nc.scalar, nc.vector]
        wo_next = [0]  # next Wo token-tile to emit

        def emit_wo(ready_cols):
            """Emit Wo tiles whose oh8/ol8 inputs are complete."""
            while wo_next[0] < NT:
                tb = wo_next[0]
                w = min(128, NPACK - tb * 128)
                if tb * 128 + w > ready_cols:
                    return
                t0 = tb * 128
                ob = obpool.tile([128, HID], f16, tag="ob", name=f"ob{tb}")
                for hc in range(HID // 512):
                    f_ps = psP.tile([128, 512], f32, tag="ps", name=f"f{tb}_{hc}")
                    n = 0
                    for oi, wi in ((0, 0), (0, 1), (1, 0)):
                        o8 = oh8 if oi == 0 else ol8
                        w8 = wo8[wi]
                        for hp in (0, 2):
                            nc.tensor.matmul(
                                f_ps[:w, :],
                                o8[:, hp : hp + 2, t0 : t0 + w],
                                w8[:, hp : hp + 2, hc * 512 : (hc + 1) * 512],
                                start=(n == 0),
                                stop=(n == 5),
                                perf_mode=DR,
                            )
                            n += 1
                    cp(eng_ob[hc % 2], ob[:w, hc * 512 : (hc + 1) * 512], f_ps[:w, :])
                    nc.sync.dma_start(
                        out[t0 : t0 + w, hc * 512 : (hc + 1) * 512],
                        ob[:w, hc * 512 : (hc + 1) * 512],
                    )
                wo_next[0] += 1

        nblk_j = []
        for s in range(NSEG):
            rem = int(nhat[s]) - (T[s] - 1) * 128
            for i in range(T[s]):
                nblk_j.append((s, i, 0 if (i < T[s] - 1 or rem == 128) else 1 + s))

        work = [(j, h) for j in range(len(nblk_j)) for h in range(QH)]
        state = {}

        def stage_a(idx):
            """scores -> exp -> diagonal mask"""
            j, h = work[idx]
            s, i, midx = nblk_j[j]
            nkb = i + 1
            jj = pbase[s] + i
            s_ps = psS.tile([128, 3, 128], f32, tag="S", name=f"s{j}_{h}")
            for ib in range(nkb):
                kb = pbase[s] + ib
                nc.tensor.matmul(
                    s_ps[:, ib, :],
                    kT[:, kb * 128 : (kb + 1) * 128],
                    qT[h][:, jj * 128 : (jj + 1) * 128],
                    start=True,
                    stop=True,
                )
            P = ppool.tile([128, 3, 128], f16, tag="P", name=f"p{j}_{h}")
            nc.scalar.activation(
                P[:, :nkb, :], s_ps[:, :nkb, :], EXP, scale=SCALE_EFF
            )
            nc.vector.tensor_mul(
                P[:, nkb - 1, :], P[:, nkb - 1, :], msk[:, midx, :]
            )
            state[idx] = P

        def stage_b(idx):
            """denominator + AV accumulation + reciprocal"""
            j, h = work[idx]
            s, i, midx = nblk_j[j]
            nkb = i + 1
            P = state[idx]
            lo = psLO.tile([128, 256], f32, tag="lo", name=f"lo{j}_{h}")
            for ib in range(nkb):
                nc.tensor.matmul(
                    lo[0:1, 128:256],
                    ones[:],
                    P[:, ib, :],
                    start=(ib == 0),
                    stop=(ib == nkb - 1),
                )
            for ib in range(nkb):
                kb = pbase[s] + ib
                nc.tensor.matmul(
                    lo[:, 0:128],
                    vA[:, kb, :],
                    P[:, ib, :],
                    start=(ib == 0),
                    stop=(ib == nkb - 1),
                )
            rc = spool.tile([1, 128], f16, tag="rc", name=f"rc{j}_{h}")
            with nc.allow_low_precision(
                reason="fp16 1/l scales fp16 outputs; ~5e-4 rel ok"
            ):
                nc.vector.reciprocal(rc[:], lo[0:1, 128:256])
            state[idx] = (lo, rc)

        def stage_c(idx):
            """1/l broadcast (PE outer product) -> normalize -> fp8 split"""
            j, h = work[idx]
            s, i, midx = nblk_j[j]
            w = min(128, int(nhat[s]) - i * 128)
            nj0 = nbase[s] + i * 128
            lo, rc = state.pop(idx)
            nc.tensor.matmul(
                lo[:, 128:256], ones1[:], rc[:], start=True, stop=True
            )
            t16 = spool.tile([128, 128], f16, tag="t16", name=f"t16{j}_{h}")
            nc.vector.tensor_mul(t16[:, :w], lo[:, 0:w], lo[:, 128 : 128 + w])
            cp(eng_oh[(j + h) % 2], oh8[:, h, nj0 : nj0 + w], t16[:, :w])
            eng_ol[(j + h) % 2].tensor_sub(
                ol8[:, h, nj0 : nj0 + w], t16[:, :w], oh8[:, h, nj0 : nj0 + w]
            )
            if h == QH - 1 and j >= 1:
                # Wo tiles fully covered by the PREVIOUS block's columns (so
                # the fp8 o-splits they read are long since written)
                sP, iP, _ = nblk_j[j - 1]
                nc_prev = nbase[sP] + iP * 128 + min(
                    128, int(nhat[sP]) - iP * 128
                )
                emit_wo(nc_prev)

        LB, LC = 2, 4
        n_work = len(work)
        for idx in range(n_work + LC):
            if idx < n_work:
                stage_a(idx)
            if LB <= idx and idx - LB < n_work:
                stage_b(idx - LB)
            if LC <= idx and idx - LC < n_work:
                stage_c(idx - LC)
        emit_wo(NPACK)

    nc.compile()
    return nc


def _get_nc(T, nhat):
    key = (T, nhat)
    if key not in _CACHE:
        _CACHE[key] = _build_nc(T, nhat)
    return _CACHE[key]


def _split8(a):
    import ml_dtypes

    e4 = ml_dtypes.float8_e4m3
    hi = a.astype(e4)
    lo = (a - hi.astype(np.float32)).astype(e4)
    return hi, lo


def kernel(hidden_states, Wq, Wk, Wv, Wo, sid, position_ids):
    global LAST_EXEC_NS, LAST_RUN_WALL_S
    import time

    from concourse.bass_utils import run_bass_kernel_spmd

    hidden = np.asarray(hidden_states, dtype=np.float32)
    Wq = np.asarray(Wq, dtype=np.float32)
    Wk = np.asarray(Wk, dtype=np.float32)
    Wv = np.asarray(Wv, dtype=np.float32)
    Wo = np.asarray(Wo, dtype=np.float32)
    sid = np.asarray(sid)
    position_ids = np.asarray(position_ids)

    T, nhat, perms, counts = _structure(sid)
    nc = _get_nc(T, nhat)

    NBLK = sum(T)
    NPACK = int(sum(nhat))
    NT = (NPACK + 127) // 128
    nbase = np.cumsum([0] + list(nhat)).tolist()
    NMASK = 1 + NSEG

    f16 = np.float16

    # constants shared by all cores
    swpn = np.zeros((128, 128), f16)
    swpn[(np.arange(128) + 64) % 128, np.arange(128)] = 1.0
    idnn = np.eye(128, dtype=f16)
    ki = np.arange(128)[:, None]
    qi = np.arange(128)[None, :]
    tri = (ki <= qi).astype(f16)
    trin = np.zeros((128, NMASK, 128), f16)
    trin[:, 0, :] = tri
    for s in range(NSEG):
        rem = int(nhat[s]) - (T[s] - 1) * 128
        trin[:, 1 + s, :] = tri * (ki < rem)

    # weights per TP group (shared across batches)
    wgrp = []
    for g in range(TP):
        wq_dev = np.ascontiguousarray(
            (SW * Wq[g * 512 : (g + 1) * 512]).T
        ).reshape(NHC, 128, QH * D)
        wk_dev = np.ascontiguousarray(
            (SW * Wk[g * 128 : (g + 1) * 128]).T
        ).reshape(NHC, 128, D)
        wv_dev = np.ascontiguousarray(
            (SW * Wv[g * 128 : (g + 1) * 128]).T
        ).reshape(NHC, 128, D)
        # wo8[p, h, n] = SWO * Wo[n, g*512 + h*128 + p]
        wo_dev = np.ascontiguousarray(
            (SWO * Wo[:, g * 512 : (g + 1) * 512]).T.reshape(QH, 128, HID)
        ).transpose(1, 0, 2)
        ws = {}
        for name, a in (("wq8", wq_dev), ("wk8", wk_dev), ("wv8", wv_dev)):
            hi, lo = _split8(np.ascontiguousarray(a.transpose(1, 0, 2)))
            ws[name + "h"], ws[name + "l"] = hi, lo
        hi, lo = _split8(np.ascontiguousarray(wo_dev))
        ws["wo8h"], ws["wo8l"] = hi, lo
        wgrp.append(ws)

    in_maps = []
    real_rows = []
    for b in range(B):
        perm = perms[b]
        n_b = counts[b]
        # n-hat-packed x with zero fill between n_b and nhat
        xs = hidden[b].T[:, perm]  # [HID, L] sorted
        xpack = np.zeros((HID, NPACK), np.float32)
        pos = np.zeros(NPACK, np.float32)
        rows = []
        off = 0
        for s in range(NSEG):
            w = int(n_b[s])
            xpack[:, nbase[s] : nbase[s] + w] = xs[:, off : off + w] * SX
            pos[nbase[s] : nbase[s] + w] = position_ids[b][
                perm[off : off + w]
            ].astype(np.float32)
            rows.append(nbase[s] + np.arange(w))
            off += w
        real_rows.append(np.concatenate(rows))

        x8h, x8l = _split8(
            np.ascontiguousarray(xpack.reshape(NHC, 128, NPACK).transpose(1, 0, 2))
        )

        inv = 1.0 / (
            THETA ** (np.arange(0, D, 2, dtype=np.float32) / np.float32(D))
        )
        fr = pos[:, None] * inv[None, :]
        emb = np.concatenate([fr, fr], axis=1)  # [NPACK, D]
        cosT = np.ascontiguousarray(np.cos(emb).T.astype(f16))
        sinT = np.sin(emb).T.astype(np.float32).copy()
        sinT[: D // 2] *= -1.0  # fold rotate_half sign
        sinT = np.ascontiguousarray(sinT.astype(f16))

        for g in range(TP):
            m = dict(
                x8h=x8h,
                x8l=x8l,
                cosd=cosT,
                sind=sinT,
                trid=trin,
                idnd=idnn,
                swpd=swpn,
            )
            m.update(wgrp[g])
            in_maps.append(m)

    t0 = time.time()
    res = run_bass_kernel_spmd(nc, in_maps, core_ids=list(range(NCORES)))
    LAST_RUN_WALL_S = time.time() - t0
    LAST_EXEC_NS = res.exec_time_ns

    full = np.empty((B, L, HID), np.float32)
    for b in range(B):
        acc = np.asarray(res.results[4 * b]["out"]).astype(np.float32)
        for g in range(1, TP):
            acc += np.asarray(res.results[4 * b + g]["out"]).astype(np.float32)
        unp = np.empty((L, HID), np.float32)
        unp[perms[b]] = acc[real_rows[b]]
        full[b] = unp * OUT_DESCALE
    return full
